# revision 2
# baseline (speedup 1.0000x reference)
"""CatNet spiking CNN on 8x TRN2 NeuronCores (data-parallel over batch N=64).

v2: conv2/conv3 stream ONE column pass per tap with the bf16 hi/lo weight
split packed into the M dimension: lhs [K, 128] has W_hi at out-cols 0-63 and
-W_lo at cols 64-127, so PSUM partitions 0-63 hold (Whi.S + X - S_own) and
64-127 hold (-Wlo.S). X-inject and the state subtract ride the PE as identity
matmuls. The spike decision is then one DVE op per (image, half):
    s = is_ge(psum_hi - thr, psum_neg_lo)   <=>  hi+lo+X-S >= thr.
This halves conv2/conv3 tensor columns vs the 2-pass hi/lo while keeping the
exact-product precision class. States are small integers -> exact in bf16.
"""
import numpy as np
import ml_dtypes

import concourse.bass as bass
import concourse.mybir as mybir
from concourse import tile
from concourse.bass_utils import run_bass_kernel_spmd
from concourse.tile_rust import add_dep_helper

F32 = mybir.dt.float32
BF16 = mybir.dt.bfloat16
BF = ml_dtypes.bfloat16
ALU = mybir.AluOpType
ACTF = mybir.ActivationFunctionType

NCORE = 8
NI = 8
T = 16
WP = 32                 # padded row stride (30 cols used; 64B-aligned bf16 rows)
HPAD = 30
PADPIX = HPAD * WP      # 960
NPIX = 784
NH = 392
TAPS = [(ky, kx) for ky in range(3) for kx in range(3)]


def _split_excess_waits(nc, maxw=1):
    """This walrus accepts only one sync-wait per instruction; hoist extras
    onto preceding same-engine nops."""
    ctr = [0]
    for bb in nc.m.functions[0].blocks:
        nl = []
        for inst in bb.instructions:
            w = list(inst.sync_info.on_wait) if inst.sync_info else []
            if len(w) > maxw:
                keep, exc = w[:maxw], w[maxw:]
                for cs in range(0, len(exc), maxw):
                    nop = mybir.InstNoOp(name=f"I-ws-{ctr[0]}", ins=[], outs=[])
                    ctr[0] += 1
                    nop.engine = inst.engine
                    nop.sync_info = mybir.SyncInfo(
                        on_wait=list(exc[cs:cs + maxw]), on_update=[])
                    nc.register_instruction(nop)
                    nl.append(nop)
                inst.sync_info = mybir.SyncInfo(
                    on_wait=list(keep), on_update=list(inst.sync_info.on_update))
            nl.append(inst)
        bb.instructions[:] = nl


def _rap(handle, offset, dims):
    """Raw access pattern on a DRAM tensor handle."""
    return bass.AP(handle, offset, [list(d) for d in dims])


def _pad3(ap_pad, nrow=128):
    """[P, 960] padded tile -> [P, 30, 32] view."""
    return ap_pad.rearrange("p (h w) -> p h w", w=WP)


def _interior(ap_pad):
    """[P, 960] padded tile -> [P, 28, 28] interior view."""
    return _pad3(ap_pad)[:, 1:29, 1:29]


def _cwin(ap_pad, ky, kx, h0):
    """Conv-tap window: out rows h0..h0+13 -> padded rows h0+ky.., cols kx.."""
    return _pad3(ap_pad)[:, h0 + ky:h0 + ky + 14, kx:kx + 28]


def _ihalf(ap_pad, e, h):
    """Interior half view [64, 14, 28] of image e in a padded pair tile."""
    v = ap_pad[64 * e:64 * e + 64, :].rearrange("p (h w) -> p h w", w=WP)
    return v[:, 1 + 14 * h:15 + 14 * h, 1:29]


def _pwin(ap_pad, e0, e1, dy, dx):
    """Pool window on partitions e0:e1: [P, 14, 14], elem (r,c) = padded
    (2r+1+dy, 2c+1+dx)."""
    v = ap_pad[e0:e1, :].rearrange("p (h w) -> p h w", w=WP)
    return v[:, 1 + dy:1 + dy + 28:2, 1 + dx:1 + dx + 28:2]


def build_nc(nt=T, ni=NI):
    nc = bass.Bass()
    qn = ni // 4
    pn = ni // 2

    xst = nc.dram_tensor("xst", [ni * nt, 27, 2, NH], BF16, kind="ExternalInput")
    w1q = nc.dram_tensor("w1q", [128, 3, 128], BF16, kind="ExternalInput")
    w2m = nc.dram_tensor("w2m", [128, 9, 128], BF16, kind="ExternalInput")
    w3m = nc.dram_tensor("w3m", [128, 9, 128], BF16, kind="ExternalInput")
    idq = nc.dram_tensor("idq", [128, 64], BF16, kind="ExternalInput")
    idxq = nc.dram_tensor("idxq", [128, 128], BF16, kind="ExternalInput")
    idsq = nc.dram_tensor("idsq", [128, 128], BF16, kind="ExternalInput")
    wf1s = nc.dram_tensor("wf1s", [98, 2, 128, 128], BF16, kind="ExternalInput")
    wf2s = nc.dram_tensor("wf2s", [128, 2, 10], BF16, kind="ExternalInput")
    thrL1 = nc.dram_tensor("thrL1", [128, nt], F32, kind="ExternalInput")
    thrH1 = nc.dram_tensor("thrH1", [128, nt], F32, kind="ExternalInput")
    thrH2 = nc.dram_tensor("thrH2", [128, nt], F32, kind="ExternalInput")
    thrF1 = nc.dram_tensor("thrF1", [128, nt], F32, kind="ExternalInput")
    bf2t = nc.dram_tensor("bf2t", [10, 1], F32, kind="ExternalInput")
    out = nc.dram_tensor("out", [ni, 10], F32, kind="ExternalOutput")
    sh3d = nc.dram_tensor("sh3d", [64, 196, ni, nt], BF16, kind="Internal")

    with tile.TileContext(nc) as tc:
        with (
            tc.tile_pool(name="wpool", bufs=1) as wp,
            tc.tile_pool(name="state", bufs=1) as stp,
            tc.tile_pool(name="scratch", bufs=1) as scp,
            tc.tile_pool(name="psum", bufs=1, space="PSUM") as psp,
        ):
            # ---- weights / thresholds ----
            w1s = wp.tile([128, 3, 128], BF16, name="w1s")
            nc.sync.dma_start(w1s[:, :, :], w1q[:, :, :])
            w2s = wp.tile([128, 9, 128], BF16, name="w2s")
            nc.sync.dma_start(w2s[:, :, :], w2m[:, :, :])
            w3s = wp.tile([128, 9, 128], BF16, name="w3s")
            nc.sync.dma_start(w3s[:, :, :], w3m[:, :, :])
            ids = wp.tile([128, 64], BF16, name="ids")
            nc.sync.dma_start(ids[:, :], idq[:, :])
            idxs = wp.tile([128, 128], BF16, name="idxs")
            nc.sync.dma_start(idxs[:, :], idxq[:, :])
            idss = wp.tile([128, 128], BF16, name="idss")
            nc.sync.dma_start(idss[:, :], idsq[:, :])
            wf2l = wp.tile([128, 2, 10], BF16, name="wf2l")
            nc.sync.dma_start(wf2l[:, :, :], wf2s[:, :, :])
            tL1 = wp.tile([128, nt], F32, name="tL1")
            nc.sync.dma_start(tL1[:, :], thrL1[:, :])
            tH1 = wp.tile([128, nt], F32, name="tH1")
            nc.sync.dma_start(tH1[:, :], thrH1[:, :])
            tH2 = wp.tile([128, nt], F32, name="tH2")
            nc.sync.dma_start(tH2[:, :], thrH2[:, :])
            tF1 = wp.tile([128, nt], F32, name="tF1")
            nc.sync.dma_start(tF1[:, :], thrF1[:, :])
            bf2l = wp.tile([10, 1], F32, name="bf2l")
            nc.sync.dma_start(bf2l[:, :], bf2t[:, :])
            wf1l = wp.tile([128, 2, 98, 128], BF16, name="wf1l")
            for hl in range(2):
                nc.sync.dma_start(
                    wf1l[:, hl, :, :],
                    _rap(wf1s, hl * 128 * 128,
                         [[128, 128], [2 * 128 * 128, 98], [1, 128]]))

            # ---- persistent states ----
            sL1 = []        # per image [128, 784]: rows 0-63 S_x1, 64-95 S_h0
            for i in range(ni):
                st = stp.tile([128, NPIX], BF16, name=f"sL1_{i}")
                nc.gpsimd.memset(st[:, :], 0.0)
                sL1.append(st)
            sH0q = []       # per quad, padded, img g at rows 32g (conv2 rhs)
            for q in range(qn):
                st = stp.tile([128, PADPIX], BF16, name=f"sH0q_{q}")
                nc.gpsimd.memset(st[:, :], 0.0)
                sH0q.append(st)
            sX1 = []        # per pair, padded, img e at rows 64e (x1 spikes)
            for p in range(pn):
                st = stp.tile([128, PADPIX], BF16, name=f"sX1_{p}")
                nc.gpsimd.memset(st[:, :], 0.0)
                sX1.append(st)
            sH1, sH2, qH3 = [], [], []
            for p in range(pn):
                s1 = stp.tile([128, PADPIX], BF16, name=f"sH1_{p}")
                nc.gpsimd.memset(s1[:, :], 0.0)
                sH1.append(s1)
                s2 = stp.tile([128, PADPIX], BF16, name=f"sH2_{p}")
                nc.gpsimd.memset(s2[:, :], 0.0)
                sH2.append(s2)
                q3 = stp.tile([128, 196, nt + 1], BF16, name=f"qH3_{p}")
                nc.gpsimd.memset(q3[:, :, :], 1.0)   # Q = S + 1
                qH3.append(q3)

            last_in_bank = {}  # chain-key -> last matmul inst of prior chain

            def chain_dep(key, first_mm):
                if key in last_in_bank:
                    add_dep_helper(first_mm.ins, last_in_bank[key].ins,
                                   sync=False, reason="psum group order")

            FdL = {}
            Fd1 = {}
            Fd2 = {}
            Fd3 = {}

            def emit_wave(chains):
                n = max(len(c) for c in chains)
                for j in range(n):
                    for c in chains:
                        if j < len(c):
                            fw, lhs, rhs, st, sp, tp, key = c[j]
                            mm = nc.tensor.matmul(fw, lhs, rhs, start=st,
                                                  stop=sp, tile_position=tp)
                            if st:
                                chain_dep(key, mm)
                            if sp:
                                last_in_bank[key] = mm

            def stage_l1(t, q):
                imgs = [4 * q + g for g in range(4)]
                # ---------- L1 ----------
                imst = scp.tile([128, NPIX], BF16, tag="imst",
                                name=f"imst_{t}_{q}", bufs=3)
                for g, i in enumerate(imgs):
                    off = (i * nt + t) * 27 * 2 * NH
                    nc.sync.dma_start(imst[32 * g:32 * g + 27, :],
                                      _rap(xst, off, [[2 * NH, 27], [1, 2 * NH]]))
                fLs = FdL
                l1_mms = {i: [] for i in imgs}
                for g, i in enumerate(imgs):
                    for h in range(2):
                        fLs[(i, h)] = psp.tile([128, 512], F32, tag="F",
                                               name=f"fL_{t}_{i}_{h}", bufs=6)
                        fw = fLs[(i, h)][:, 0:NH]
                        rh = imst[32 * g:32 * g + 27, NH * h:NH * h + NH]
                        for v in range(3):
                            l1_mms[i].append(
                                (fw, w1s[32 * g:32 * g + 27, v, :], rh,
                                 v == 0, v == 2, (32 * g, 0), ("L", i, h)))
                # round-robin across images for row-group diversity
                for j in range(6):
                    for g, i in enumerate(imgs):
                        fw, lhs, rh, st, sp, tp, key = l1_mms[i][j]
                        mm = nc.tensor.matmul(fw, lhs, rh, start=st, stop=sp,
                                              tile_position=tp)
                        if st:
                            chain_dep(key, mm)
                        if sp:
                            last_in_bank[key] = mm
                for g, i in enumerate(imgs):
                    sL = scp.tile([128, NPIX], BF16, tag="s",
                                  name=f"s_{t}_{i}", bufs=6)
                    for h in range(2):
                        nc.vector.scalar_tensor_tensor(
                            sL[:, NH * h:NH * h + NH],
                            fLs[(i, h)][:, 0:NH],
                            tL1[:, t:t + 1],
                            sL1[i][:, NH * h:NH * h + NH],
                            op0=ALU.subtract, op1=ALU.is_ge)
                    nc.vector.tensor_tensor(
                        sL1[i][:, :], sL1[i][:, :], sL[:, :], ALU.add)
                    # maintain padded quad h0 copy on ScalarE (partition shift)
                    nc.scalar.copy(
                        _interior(sH0q[q][32 * g:32 * g + 32, :]),
                        sL1[i][64:96, :].rearrange("p (h w) -> p h w", w=28))
                    # maintain padded x1 pair copy (img e at rows 64e)
                    p_ = i // 2
                    e_ = i % 2
                    nc.scalar.copy(
                        _interior(sX1[p_][64 * e_:64 * e_ + 64, :]),
                        sL1[i][0:64, :].rearrange("p (h w) -> p h w", w=28))

            def stage_h1_mm(t, q):
                # ---------- h1: conv2(S_h0) hi|-lo in M + X - S on PE ----------
                f1s = Fd1
                for pp in range(2):
                    for e in range(2):
                        i = 4 * q + 2 * pp + e
                        for h in range(2):
                            f1s[(i, h)] = psp.tile([128, 512], F32, tag="F",
                                                   name=f"f1_{t}_{i}_{h}",
                                                   bufs=6)

                def h1_chain_list(pp, h, e):
                    p = 2 * q + pp
                    i = 4 * q + 2 * pp + e
                    g = 2 * pp + e
                    fw = f1s[(i, h)][:, 0:NH]
                    key = ("h1", i, h)
                    mms = [
                        (fw, idxs[64 * e:64 * e + 64, :], _ihalf(sX1[p], e, h),
                         True, False, (64 * e, 0), key),
                        (fw, idss[64 * e:64 * e + 64, :], _ihalf(sH1[p], e, h),
                         False, False, (64 * e, 0), key),
                    ]
                    for k, (ky, kx) in enumerate(TAPS):
                        rhs = _cwin(sH0q[q][32 * g:32 * g + 32, :], ky, kx,
                                    14 * h)
                        mms.append((fw, w2s[32 * g:32 * g + 32, k, :], rhs,
                                    False, k == 8, (32 * g, 0), key))
                    return mms

                waves = ([h1_chain_list(0, 0, 0), h1_chain_list(0, 1, 1),
                          h1_chain_list(1, 0, 0), h1_chain_list(1, 1, 1)],
                         [h1_chain_list(0, 0, 1), h1_chain_list(0, 1, 0),
                          h1_chain_list(1, 0, 1), h1_chain_list(1, 1, 0)])

                return waves

            def stage_h1_post(t, q):
                f1s = Fd1
                for pp in range(2):
                    p = 2 * q + pp
                    sP = scp.tile([128, NPIX], BF16, tag="s",
                                  name=f"sh1_{t}_{p}", bufs=6)
                    for e in range(2):
                        i = 4 * q + 2 * pp + e
                        lo = 64 * (1 - e)
                        hi = 64 * e
                        for h in range(2):
                            D = scp.tile([128, NH], F32, tag="d",
                                         name=f"d1_{t}_{i}_{h}", bufs=6)
                            nc.scalar.activation(
                                D[hi:hi + 64, :], f1s[(i, h)][lo:lo + 64, 0:NH],
                                ACTF.Identity, bias=tH1[hi:hi + 64, t:t + 1],
                                scale=1.0)
                            nc.vector.scalar_tensor_tensor(
                                sP[hi:hi + 64, NH * h:NH * h + NH],
                                f1s[(i, h)][hi:hi + 64, 0:NH],
                                0.0,
                                D[hi:hi + 64, :],
                                op0=ALU.add, op1=ALU.is_ge)
                    nc.vector.tensor_tensor(
                        _interior(sH1[p][:, :]), _interior(sH1[p][:, :]),
                        sP[:, :].rearrange("p (h w) -> p h w", w=28), ALU.add)

            def stage_h2_mm(t, q):
                # ---------- h2: conv3(S_h1) hi|-lo in M, - S on PE ----------
                f2s = Fd2
                for pp in range(2):
                    for e in range(2):
                        i = 4 * q + 2 * pp + e
                        for h in range(2):
                            f2s[(i, h)] = psp.tile([128, 512], F32, tag="F",
                                                   name=f"f2_{t}_{i}_{h}",
                                                   bufs=6)

                def h2_chain_list(pp, h, e):
                    p = 2 * q + pp
                    i = 4 * q + 2 * pp + e
                    fw = f2s[(i, h)][:, 0:NH]
                    key = ("h2", i, h)
                    mms = [(fw, idss[64 * e:64 * e + 64, :],
                            _ihalf(sH2[p], e, h),
                            True, False, (64 * e, 0), key)]
                    for k, (ky, kx) in enumerate(TAPS):
                        rhs = _cwin(sH1[p][64 * e:64 * e + 64, :], ky, kx,
                                    14 * h)
                        mms.append((fw, w3s[64 * e:64 * e + 64, k, :], rhs,
                                    False, k == 8, (64 * e, 0), key))
                    return mms

                waves = ([h2_chain_list(0, 0, 0), h2_chain_list(0, 1, 1),
                          h2_chain_list(1, 0, 0), h2_chain_list(1, 1, 1)],
                         [h2_chain_list(0, 0, 1), h2_chain_list(0, 1, 0),
                          h2_chain_list(1, 0, 1), h2_chain_list(1, 1, 0)])

                return waves

            def stage_h2_post(t, q):
                f2s = Fd2
                for pp in range(2):
                    p = 2 * q + pp
                    sP = scp.tile([128, NPIX], BF16, tag="s",
                                  name=f"sh2_{t}_{p}", bufs=6)
                    for e in range(2):
                        i = 4 * q + 2 * pp + e
                        lo = 64 * (1 - e)
                        hi = 64 * e
                        for h in range(2):
                            D = scp.tile([128, NH], F32, tag="d",
                                         name=f"d2_{t}_{i}_{h}", bufs=6)
                            nc.scalar.activation(
                                D[hi:hi + 64, :], f2s[(i, h)][lo:lo + 64, 0:NH],
                                ACTF.Identity, bias=tH2[hi:hi + 64, t:t + 1],
                                scale=1.0)
                            nc.vector.scalar_tensor_tensor(
                                sP[hi:hi + 64, NH * h:NH * h + NH],
                                f2s[(i, h)][hi:hi + 64, 0:NH],
                                0.0,
                                D[hi:hi + 64, :],
                                op0=ALU.add, op1=ALU.is_ge)
                    nc.vector.tensor_tensor(
                        _interior(sH2[p][:, :]), _interior(sH2[p][:, :]),
                        sP[:, :].rearrange("p (h w) -> p h w", w=28), ALU.add)

            def stage_h3_mm(t, q):
                # ---------- h3: 2x2 sum pool (x1.1 in compare) ----------
                f3s = Fd3
                for pp in range(2):
                    p = 2 * q + pp
                    f3s[p] = psp.tile([128, 512], F32, tag="F3",
                                      name=f"f3_{t}_{p}", bufs=2)

                def pool_chain_list(pp, e):
                    p = 2 * q + pp
                    col = 64 * e
                    fw = f3s[p][col:col + 64, 0:196]
                    key = ("h3", p)
                    mms = []
                    for wi, (dy, dx) in enumerate(
                            ((0, 0), (0, 1), (1, 0), (1, 1))):
                        rhs = _pwin(sH2[p][:, :], 64 * e, 64 * e + 64, dy, dx)
                        mms.append((fw, ids[64 * e:64 * e + 64, :], rhs,
                                    wi == 0, wi == 3, (64 * e, col), key))
                    return mms

                waves = ([pool_chain_list(0, 0), pool_chain_list(1, 1)],
                         [pool_chain_list(0, 1), pool_chain_list(1, 0)])

                return waves

            def stage_h3_post(t, q):
                f3s = Fd3
                for pp in range(2):
                    p = 2 * q + pp
                    sP = scp.tile([128, NPIX], BF16, tag="s",
                                  name=f"sh3_{t}_{p}", bufs=6)
                    nc.vector.scalar_tensor_tensor(
                        sP[:, 0:196], f3s[p][:, 0:196], 1.1,
                        qH3[p][:, :, t], op0=ALU.mult, op1=ALU.is_ge)
                    nc.vector.tensor_tensor(
                        qH3[p][:, :, t + 1], qH3[p][:, :, t],
                        sP[:, 0:196], ALU.add)

            # ================= time loop: L1(t+1) interleaves into t ====
            for q in range(qn):
                stage_l1(0, q)
            for t in range(nt):
                for q in range(qn):
                    for wv in stage_h1_mm(t, q):
                        emit_wave(wv)
                    stage_h1_post(t, q)
                if t + 1 < nt:
                    for q in range(qn):
                        stage_l1(t + 1, q)
                for q in range(qn):
                    for wv in stage_h2_mm(t, q):
                        emit_wave(wv)
                    stage_h2_post(t, q)
                for q in range(qn):
                    for wv in stage_h3_mm(t, q):
                        emit_wave(wv)
                    stage_h3_post(t, q)

            # ---- ship Q trajectories to DRAM ----
            for p in range(pn):
                for e in range(2):
                    i = 2 * p + e
                    dst = _rap(sh3d, i * nt,
                               [[196 * ni * nt, 64], [ni * nt, 196], [1, nt]])
                    nc.sync.dma_start(dst, qH3[p][64 * e:64 * e + 64, :, 1:nt + 1])

            # ---------- fc1 ----------
            ff1 = psp.tile([128, 512], F32, tag="F3", name="ff1", bufs=2)
            CH = 14
            for c0 in range(0, 98, CH):
                rt = scp.tile([128, CH, ni * nt], BF16, tag="rf1t",
                              name=f"rf1t_{c0}", bufs=3)
                for j in range(2):
                    src = _rap(sh3d, (2 * c0 + j) * ni * nt,
                               [[196 * ni * nt, 64], [2 * ni * nt, CH],
                                [1, ni * nt]])
                    nc.sync.dma_start(rt[64 * j:64 * j + 64, :, :], src)
                for dx in range(CH):
                    px = c0 + dx
                    for hl in range(2):
                        nc.tensor.matmul(ff1[:, 0:ni * nt],
                                         wf1l[:, hl, px, :], rt[:, dx, :],
                                         start=(px == 0 and hl == 0),
                                         stop=(px == 97 and hl == 1))

            # f1 spike scan (Q_h3 includes +1 offset; folded into thrF1)
            sf1 = stp.tile([128, ni], BF16, name="sf1")
            nc.gpsimd.memset(sf1[:, :], 0.0)
            for t in range(nt):
                sPf = scp.tile([128, ni], BF16, tag="sf", name=f"sf_{t}", bufs=2)
                nc.vector.scalar_tensor_tensor(
                    sPf[:, :],
                    ff1[:, 0:ni * nt].rearrange("p (n t) -> p n t", t=nt)[:, :, t],
                    tF1[:, t:t + 1], sf1[:, :],
                    op0=ALU.subtract, op1=ALU.is_ge)
                nc.vector.tensor_tensor(sf1[:, :], sf1[:, :], sPf[:, :], ALU.add)

            # ---------- fc2 + readout ----------
            ff2 = psp.tile([128, 512], F32, tag="F3", name="ff2", bufs=2)
            for hl in range(2):
                nc.tensor.matmul(ff2[0:10, 0:ni], wf2l[:, hl, :], sf1[:, :],
                                 start=(hl == 0), stop=(hl == 1))
            osb = scp.tile([10, ni], F32, tag="osb", name="osb")
            nc.scalar.activation(osb[:, :], ff2[0:10, 0:ni], ACTF.Identity,
                                 bias=bf2l[:, :], scale=1.0 / nt)
            nc.sync.dma_start(out[:, :].rearrange("n o -> o n"), osb[:, :])

    _split_excess_waits(nc)
    return nc


# ---------------- host side ----------------

def _split(a):
    hi = np.asarray(a, np.float64).astype(BF)
    lo = (np.asarray(a, np.float64) - hi.astype(np.float64)).astype(BF)
    return hi, lo


def _split3(a):
    a = np.asarray(a, np.float64)
    p0 = a.astype(BF)
    r = a - p0.astype(np.float64)
    p1 = r.astype(BF)
    p2 = (r - p1.astype(np.float64)).astype(BF)
    return p0, p1, p2


def _prep_shared(w1a, b1a, w1, b1, w2, b2, w3, b3, wf1, bf1, wf2, bf2, nt=T):
    d = {}
    # L1 lhsT [9, 128]: cols 0-63 w1a, 64-95 w1, 96-127 zero; rows k=3ky+kx
    # 3-term split W = W0+W1+W2; K=27 stacked variants to cover products with
    # the 3-term x split (X0,X1,X2): [W0;W1;W2], [W1;W0;0], [W2;0;W0]
    l1 = np.zeros((9, 128), np.float64)
    l1[:, 0:64] = w1a.reshape(64, 9).T
    l1[:, 64:96] = w1.reshape(32, 9).T
    W0, W1, W2 = _split3(l1)
    Z = np.zeros_like(W0)
    variants = [np.concatenate([W0, W1, W2]), np.concatenate([W1, W0, Z]),
                np.concatenate([W2, Z, W0])]
    w1qa = np.zeros((128, 3, 128), BF)
    for g in range(4):
        for v in range(3):
            w1qa[32 * g:32 * g + 27, v, :] = variants[v]
    d["w1q"] = w1qa
    # conv2 [tap, c, m]: hi at cols 0-63, -lo at cols 64-127, 4 row bases
    a2 = np.transpose(w2.reshape(64, 32, 3, 3), (2, 3, 1, 0)).reshape(9, 32, 64)
    h2v, l2v = _split(a2)
    w2ma = np.zeros((128, 9, 128), BF)
    for g in range(4):
        e = g % 2
        hi, lo = 64 * e, 64 * (1 - e)
        w2ma[32 * g:32 * g + 32, :, hi:hi + 64] = np.transpose(h2v, (1, 0, 2))
        w2ma[32 * g:32 * g + 32, :, lo:lo + 64] = -np.transpose(l2v, (1, 0, 2))
    d["w2m"] = w2ma
    a3 = np.transpose(w3.reshape(64, 64, 3, 3), (2, 3, 1, 0)).reshape(9, 64, 64)
    h3v, l3v = _split(a3)
    w3ma = np.zeros((128, 9, 128), BF)
    for e in range(2):
        hi, lo = 64 * e, 64 * (1 - e)
        w3ma[64 * e:64 * e + 64, :, hi:hi + 64] = np.transpose(h3v, (1, 0, 2))
        w3ma[64 * e:64 * e + 64, :, lo:lo + 64] = -np.transpose(l3v, (1, 0, 2))
    d["w3m"] = w3ma
    idqa = np.zeros((128, 64), BF)
    for e in range(2):
        idqa[64 * e:64 * e + 64, :] = np.eye(64, dtype=BF)
    d["idq"] = idqa
    idxa = np.zeros((128, 128), BF)
    idsa = np.zeros((128, 128), BF)
    for e in range(2):
        idxa[64 * e:64 * e + 64, 64 * e:64 * e + 64] = np.eye(64, dtype=BF)
        idsa[64 * e:64 * e + 64, 64 * e:64 * e + 64] = -np.eye(64, dtype=BF)
    d["idxq"] = idxa
    d["idsq"] = idsa
    # fc1 tiles [pxh, hl, row=64j+c, m]; fc1 consumes h3 spikes (no 1.1)
    wf1p = np.asarray(wf1, np.float64)                  # [128, 64, 14, 14]
    wf1f = wf1p.reshape(128, 64, 196)                   # px = 14*h + w
    wf1sa = np.zeros((98, 2, 128, 128), BF)
    hi1, lo1 = _split(wf1f)
    for pxh in range(98):
        for j in range(2):
            px = 2 * pxh + j
            wf1sa[pxh, 0, 64 * j:64 * j + 64, :] = hi1[:, :, px].T
            wf1sa[pxh, 1, 64 * j:64 * j + 64, :] = lo1[:, :, px].T
    d["wf1s"] = wf1sa
    h2v_, l2v_ = _split(np.asarray(wf2, np.float64).T)    # [128, 10]
    wf2sa = np.zeros((128, 2, 10), BF)
    wf2sa[:, 0, :] = h2v_
    wf2sa[:, 1, :] = l2v_
    d["wf2s"] = wf2sa
    # thresholds [128, nt]
    tsteps = np.arange(1, nt + 1)
    thrL1 = np.ones((128, nt), np.float32)
    thrL1[0:64] = 1.0 - tsteps[None, :] * np.asarray(b1a, np.float64)[:, None]
    thrL1[64:96] = 1.0 - tsteps[None, :] * np.asarray(b1, np.float64)[:, None]
    d["thrL1"] = thrL1
    thrH1 = np.ones((128, nt), np.float32)
    b2d = np.asarray(b2, np.float64)
    thrH1[0:64] = 1.0 - tsteps[None, :] * b2d[:, None]
    thrH1[64:128] = 1.0 - tsteps[None, :] * b2d[:, None]
    d["thrH1"] = thrH1
    thrH2 = np.ones((128, nt), np.float32)
    b3d = np.asarray(b3, np.float64)
    thrH2[0:64] = 1.0 - tsteps[None, :] * b3d[:, None]
    thrH2[64:128] = 1.0 - tsteps[None, :] * b3d[:, None]
    d["thrH2"] = thrH2
    # f1: (F_comp = wf1p . Q) >= S + 1 + rowsum(wf1p) - (t+1)*bf1
    rs = wf1f.sum(axis=(1, 2))                          # [128]
    thrF1 = np.zeros((128, nt), np.float32)
    thrF1[:, :] = (1.0 + rs[:, None]
                   - tsteps[None, :] * np.asarray(bf1, np.float64)[:, None])
    d["thrF1"] = thrF1
    d["bf2t"] = np.asarray(bf2, np.float32).reshape(10, 1)
    return d


def _prep_x(xc, nt=T):
    """xc [ni, 1, 28, 28, nt] -> host im2col of the 3-term-split cumulative
    input, stacked [ni*nt, 27, 2, 392] = ([X0;X1;X2], half, px)."""
    ni = xc.shape[0]
    X = np.cumsum(np.asarray(xc, np.float64), axis=-1)[:, 0]   # [ni, 28, 28, nt]
    X = np.moveaxis(X, -1, 1)                                  # [ni, nt, 28, 28]
    pad = np.zeros((ni * nt, 30, 30), np.float64)
    pad[:, 1:29, 1:29] = X.reshape(ni * nt, 28, 28)
    col = np.zeros((ni * nt, 9, 2, NH), np.float64)
    for k, (ky, kx) in enumerate(TAPS):
        for h in range(2):
            col[:, k, h, :] = pad[:, 14 * h + ky:14 * h + ky + 14,
                                  kx:kx + 28].reshape(ni * nt, NH)
    x0, x1, x2 = _split3(col)
    return np.concatenate([x0, x1, x2], axis=1)  # [ni*nt, 27, 2, NH]


_NC_CACHE = {}


def kernel(x, w1a, b1a, w1, b1, w2, b2, w3, b3, wf1, bf1, wf2, bf2):
    x = np.asarray(x)
    n_total = x.shape[0]
    ni = n_total // NCORE
    key = (ni, T)
    if key not in _NC_CACHE:
        _NC_CACHE[key] = build_nc(T, ni)
    nc = _NC_CACHE[key]

    shared = _prep_shared(w1a, b1a, w1, b1, w2, b2, w3, b3, wf1, bf1, wf2, bf2)
    in_maps = []
    for c in range(NCORE):
        m = dict(shared)
        m["xst"] = _prep_x(x[c * ni:(c + 1) * ni])
        in_maps.append(m)

    res = run_bass_kernel_spmd(nc, in_maps, list(range(NCORE))).results
    return np.concatenate([res[c]["out"] for c in range(NCORE)], axis=0)


# revision 3
# speedup vs baseline: 1.0338x; 1.0338x over previous
"""CatNet spiking CNN on 8x TRN2 NeuronCores (data-parallel over batch N=64).

v2: conv2/conv3 stream ONE column pass per tap with the bf16 hi/lo weight
split packed into the M dimension: lhs [K, 128] has W_hi at out-cols 0-63 and
-W_lo at cols 64-127, so PSUM partitions 0-63 hold (Whi.S + X - S_own) and
64-127 hold (-Wlo.S). X-inject and the state subtract ride the PE as identity
matmuls. The spike decision is then one DVE op per (image, half):
    s = is_ge(psum_hi - thr, psum_neg_lo)   <=>  hi+lo+X-S >= thr.
This halves conv2/conv3 tensor columns vs the 2-pass hi/lo while keeping the
exact-product precision class. States are small integers -> exact in bf16.
"""
import numpy as np
import ml_dtypes

import concourse.bass as bass
import concourse.mybir as mybir
from concourse import tile
from concourse.bass_utils import run_bass_kernel_spmd
from concourse.tile_rust import add_dep_helper

F32 = mybir.dt.float32
BF16 = mybir.dt.bfloat16
BF = ml_dtypes.bfloat16
ALU = mybir.AluOpType
ACTF = mybir.ActivationFunctionType

NCORE = 8
NI = 8
T = 16
WP = 32                 # padded row stride (30 cols used; 64B-aligned bf16 rows)
HPAD = 30
PADPIX = HPAD * WP      # 960
NPIX = 784
NH = 392
TAPS = [(ky, kx) for ky in range(3) for kx in range(3)]


def _split_excess_waits(nc, maxw=1):
    """This walrus accepts only one sync-wait per instruction; hoist extras
    onto preceding same-engine nops."""
    ctr = [0]
    for bb in nc.m.functions[0].blocks:
        nl = []
        for inst in bb.instructions:
            w = list(inst.sync_info.on_wait) if inst.sync_info else []
            if len(w) > maxw:
                keep, exc = w[:maxw], w[maxw:]
                for cs in range(0, len(exc), maxw):
                    nop = mybir.InstNoOp(name=f"I-ws-{ctr[0]}", ins=[], outs=[])
                    ctr[0] += 1
                    nop.engine = inst.engine
                    nop.sync_info = mybir.SyncInfo(
                        on_wait=list(exc[cs:cs + maxw]), on_update=[])
                    nc.register_instruction(nop)
                    nl.append(nop)
                inst.sync_info = mybir.SyncInfo(
                    on_wait=list(keep), on_update=list(inst.sync_info.on_update))
            nl.append(inst)
        bb.instructions[:] = nl


def _rap(handle, offset, dims):
    """Raw access pattern on a DRAM tensor handle."""
    return bass.AP(handle, offset, [list(d) for d in dims])


def _pad3(ap_pad, nrow=128):
    """[P, 960] padded tile -> [P, 30, 32] view."""
    return ap_pad.rearrange("p (h w) -> p h w", w=WP)


def _interior(ap_pad):
    """[P, 960] padded tile -> [P, 28, 28] interior view."""
    return _pad3(ap_pad)[:, 1:29, 1:29]


def _cwin(ap_pad, ky, kx, h0):
    """Conv-tap window: out rows h0..h0+13 -> padded rows h0+ky.., cols kx.."""
    return _pad3(ap_pad)[:, h0 + ky:h0 + ky + 14, kx:kx + 28]


def _ihalf(ap_pad, e, h):
    """Interior half view [64, 14, 28] of image e in a padded pair tile."""
    v = ap_pad[64 * e:64 * e + 64, :].rearrange("p (h w) -> p h w", w=WP)
    return v[:, 1 + 14 * h:15 + 14 * h, 1:29]


def _pwin(ap_pad, e0, e1, dy, dx):
    """Pool window on partitions e0:e1: [P, 14, 14], elem (r,c) = padded
    (2r+1+dy, 2c+1+dx)."""
    v = ap_pad[e0:e1, :].rearrange("p (h w) -> p h w", w=WP)
    return v[:, 1 + dy:1 + dy + 28:2, 1 + dx:1 + dx + 28:2]


def build_nc(nt=T, ni=NI):
    nc = bass.Bass()
    qn = ni // 4
    pn = ni // 2

    xst = nc.dram_tensor("xst", [ni * nt, 27, 2, NH], BF16, kind="ExternalInput")
    w1q = nc.dram_tensor("w1q", [128, 3, 128], BF16, kind="ExternalInput")
    w2m = nc.dram_tensor("w2m", [128, 9, 128], BF16, kind="ExternalInput")
    w3m = nc.dram_tensor("w3m", [128, 9, 128], BF16, kind="ExternalInput")
    idq = nc.dram_tensor("idq", [128, 64], BF16, kind="ExternalInput")
    idxq = nc.dram_tensor("idxq", [128, 128], BF16, kind="ExternalInput")
    idsq = nc.dram_tensor("idsq", [128, 128], BF16, kind="ExternalInput")
    wf1s = nc.dram_tensor("wf1s", [98, 2, 128, 128], BF16, kind="ExternalInput")
    wf2s = nc.dram_tensor("wf2s", [128, 2, 10], BF16, kind="ExternalInput")
    thrL1 = nc.dram_tensor("thrL1", [128, nt], F32, kind="ExternalInput")
    thrH1 = nc.dram_tensor("thrH1", [128, nt], F32, kind="ExternalInput")
    thrH2 = nc.dram_tensor("thrH2", [128, nt], F32, kind="ExternalInput")
    thrF1 = nc.dram_tensor("thrF1", [128, nt], F32, kind="ExternalInput")
    bf2t = nc.dram_tensor("bf2t", [10, 1], F32, kind="ExternalInput")
    out = nc.dram_tensor("out", [ni, 10], F32, kind="ExternalOutput")
    sh3d = nc.dram_tensor("sh3d", [64, 196, ni, nt], BF16, kind="Internal")

    with tile.TileContext(nc) as tc:
        with (
            tc.tile_pool(name="wpool", bufs=1) as wp,
            tc.tile_pool(name="state", bufs=1) as stp,
            tc.tile_pool(name="scratch", bufs=1) as scp,
            tc.tile_pool(name="psum", bufs=1, space="PSUM") as psp,
        ):
            # ---- weights / thresholds ----
            w1s = wp.tile([128, 3, 128], BF16, name="w1s")
            nc.sync.dma_start(w1s[:, :, :], w1q[:, :, :])
            w2s = wp.tile([128, 9, 128], BF16, name="w2s")
            nc.sync.dma_start(w2s[:, :, :], w2m[:, :, :])
            w3s = wp.tile([128, 9, 128], BF16, name="w3s")
            nc.sync.dma_start(w3s[:, :, :], w3m[:, :, :])
            ids = wp.tile([128, 64], BF16, name="ids")
            nc.sync.dma_start(ids[:, :], idq[:, :])
            idxs = wp.tile([128, 128], BF16, name="idxs")
            nc.sync.dma_start(idxs[:, :], idxq[:, :])
            idss = wp.tile([128, 128], BF16, name="idss")
            nc.sync.dma_start(idss[:, :], idsq[:, :])
            wf2l = wp.tile([128, 2, 10], BF16, name="wf2l")
            nc.sync.dma_start(wf2l[:, :, :], wf2s[:, :, :])
            tL1 = wp.tile([128, nt], F32, name="tL1")
            nc.sync.dma_start(tL1[:, :], thrL1[:, :])
            tH1 = wp.tile([128, nt], F32, name="tH1")
            nc.sync.dma_start(tH1[:, :], thrH1[:, :])
            tH2 = wp.tile([128, nt], F32, name="tH2")
            nc.sync.dma_start(tH2[:, :], thrH2[:, :])
            tF1 = wp.tile([128, nt], F32, name="tF1")
            nc.sync.dma_start(tF1[:, :], thrF1[:, :])
            bf2l = wp.tile([10, 1], F32, name="bf2l")
            nc.sync.dma_start(bf2l[:, :], bf2t[:, :])
            wf1l = wp.tile([128, 2, 98, 128], BF16, name="wf1l")

            # ---- persistent states ----
            sL1 = []        # per image [128, 784]: rows 0-63 S_x1, 64-95 S_h0
            for i in range(ni):
                st = stp.tile([128, NPIX], BF16, name=f"sL1_{i}")
                nc.gpsimd.memset(st[:, :], 0.0)
                sL1.append(st)
            sH0q = []       # per quad, padded, img g at rows 32g (conv2 rhs)
            for q in range(qn):
                st = stp.tile([128, PADPIX], BF16, name=f"sH0q_{q}")
                nc.gpsimd.memset(st[:, :], 0.0)
                sH0q.append(st)
            sX1 = []        # per pair, padded, img e at rows 64e (x1 spikes)
            for p in range(pn):
                st = stp.tile([128, PADPIX], BF16, name=f"sX1_{p}")
                nc.gpsimd.memset(st[:, :], 0.0)
                sX1.append(st)
            sH1, sH2, qH3 = [], [], []
            for p in range(pn):
                s1 = stp.tile([128, PADPIX], BF16, name=f"sH1_{p}")
                nc.gpsimd.memset(s1[:, :], 0.0)
                sH1.append(s1)
                s2 = stp.tile([128, PADPIX], BF16, name=f"sH2_{p}")
                nc.gpsimd.memset(s2[:, :], 0.0)
                sH2.append(s2)
                q3 = stp.tile([128, 196, nt + 1], BF16, name=f"qH3_{p}")
                nc.gpsimd.memset(q3[:, :, :], 1.0)   # Q = S + 1
                qH3.append(q3)

            last_in_bank = {}  # chain-key -> last matmul inst of prior chain

            def chain_dep(key, first_mm):
                if key in last_in_bank:
                    add_dep_helper(first_mm.ins, last_in_bank[key].ins,
                                   sync=False, reason="psum group order")

            FdL = {}
            Fd1 = {}
            Fd2 = {}
            Fd3 = {}

            def emit_wave(chains):
                n = max(len(c) for c in chains)
                for j in range(n):
                    for c in chains:
                        if j < len(c):
                            fw, lhs, rhs, st, sp, tp, key = c[j]
                            mm = nc.tensor.matmul(fw, lhs, rhs, start=st,
                                                  stop=sp, tile_position=tp)
                            if st:
                                chain_dep(key, mm)
                            if sp:
                                last_in_bank[key] = mm

            def stage_l1(t, q):
                imgs = [4 * q + g for g in range(4)]
                # ---------- L1 ----------
                imst = scp.tile([128, NPIX], BF16, tag="imst",
                                name=f"imst_{t}_{q}", bufs=3)
                for g, i in enumerate(imgs):
                    off = (i * nt + t) * 27 * 2 * NH
                    nc.sync.dma_start(imst[32 * g:32 * g + 27, :],
                                      _rap(xst, off, [[2 * NH, 27], [1, 2 * NH]]))
                fLs = FdL
                l1_mms = {i: [] for i in imgs}
                for g, i in enumerate(imgs):
                    for h in range(2):
                        fLs[(i, h)] = psp.tile([128, 512], F32, tag="F",
                                               name=f"fL_{t}_{i}_{h}", bufs=6)
                        fw = fLs[(i, h)][:, 0:NH]
                        rh = imst[32 * g:32 * g + 27, NH * h:NH * h + NH]
                        for v in range(3):
                            l1_mms[i].append(
                                (fw, w1s[32 * g:32 * g + 27, v, :], rh,
                                 v == 0, v == 2, (32 * g, 0), ("L", i, h)))
                # round-robin across images for row-group diversity
                for j in range(6):
                    for g, i in enumerate(imgs):
                        fw, lhs, rh, st, sp, tp, key = l1_mms[i][j]
                        mm = nc.tensor.matmul(fw, lhs, rh, start=st, stop=sp,
                                              tile_position=tp)
                        if st:
                            chain_dep(key, mm)
                        if sp:
                            last_in_bank[key] = mm
                for g, i in enumerate(imgs):
                    sL = scp.tile([128, NPIX], BF16, tag="s",
                                  name=f"s_{t}_{i}", bufs=8)
                    for h in range(2):
                        nc.vector.scalar_tensor_tensor(
                            sL[:, NH * h:NH * h + NH],
                            fLs[(i, h)][:, 0:NH],
                            tL1[:, t:t + 1],
                            sL1[i][:, NH * h:NH * h + NH],
                            op0=ALU.subtract, op1=ALU.is_ge)
                    nc.vector.tensor_tensor(
                        sL1[i][:, :], sL1[i][:, :], sL[:, :], ALU.add)
                    # maintain padded quad h0 copy on ScalarE (partition shift)
                    nc.scalar.copy(
                        _interior(sH0q[q][32 * g:32 * g + 32, :]),
                        sL1[i][64:96, :].rearrange("p (h w) -> p h w", w=28))
                    # maintain padded x1 pair copy (img e at rows 64e)
                    p_ = i // 2
                    e_ = i % 2
                    nc.scalar.copy(
                        _interior(sX1[p_][64 * e_:64 * e_ + 64, :]),
                        sL1[i][0:64, :].rearrange("p (h w) -> p h w", w=28))

            def stage_h1_mm(t, q):
                # ---------- h1: conv2(S_h0) hi|-lo in M + X - S on PE ----------
                f1s = Fd1
                for pp in range(2):
                    for e in range(2):
                        i = 4 * q + 2 * pp + e
                        for h in range(2):
                            f1s[(i, h)] = psp.tile([128, 512], F32, tag="F",
                                                   name=f"f1_{t}_{i}_{h}",
                                                   bufs=6)

                def h1_chain_list(pp, h, e):
                    p = 2 * q + pp
                    i = 4 * q + 2 * pp + e
                    g = 2 * pp + e
                    fw = f1s[(i, h)][:, 0:NH]
                    key = ("h1", i, h)
                    mms = [
                        (fw, idxs[64 * e:64 * e + 64, :], _ihalf(sX1[p], e, h),
                         True, False, (64 * e, 0), key),
                        (fw, idss[64 * e:64 * e + 64, :], _ihalf(sH1[p], e, h),
                         False, False, (64 * e, 0), key),
                    ]
                    for k, (ky, kx) in enumerate(TAPS):
                        rhs = _cwin(sH0q[q][32 * g:32 * g + 32, :], ky, kx,
                                    14 * h)
                        mms.append((fw, w2s[32 * g:32 * g + 32, k, :], rhs,
                                    False, k == 8, (32 * g, 0), key))
                    return mms

                waves = ([h1_chain_list(0, 0, 0), h1_chain_list(0, 1, 1),
                          h1_chain_list(1, 0, 0), h1_chain_list(1, 1, 1)],
                         [h1_chain_list(0, 0, 1), h1_chain_list(0, 1, 0),
                          h1_chain_list(1, 0, 1), h1_chain_list(1, 1, 0)])

                return waves

            def stage_h1_post(t, q):
                f1s = Fd1
                for pp in range(2):
                    p = 2 * q + pp
                    sP = scp.tile([128, NPIX], BF16, tag="s",
                                  name=f"sh1_{t}_{p}", bufs=8)
                    for e in range(2):
                        i = 4 * q + 2 * pp + e
                        lo = 64 * (1 - e)
                        hi = 64 * e
                        for h in range(2):
                            D = scp.tile([128, NH], F32, tag="d",
                                         name=f"d1_{t}_{i}_{h}", bufs=8)
                            nc.scalar.activation(
                                D[hi:hi + 64, :], f1s[(i, h)][lo:lo + 64, 0:NH],
                                ACTF.Identity, bias=tH1[hi:hi + 64, t:t + 1],
                                scale=1.0)
                            nc.vector.scalar_tensor_tensor(
                                sP[hi:hi + 64, NH * h:NH * h + NH],
                                f1s[(i, h)][hi:hi + 64, 0:NH],
                                0.0,
                                D[hi:hi + 64, :],
                                op0=ALU.add, op1=ALU.is_ge)
                    nc.vector.tensor_tensor(
                        _interior(sH1[p][:, :]), _interior(sH1[p][:, :]),
                        sP[:, :].rearrange("p (h w) -> p h w", w=28), ALU.add)

            def stage_h2_mm(t, q):
                # ---------- h2: conv3(S_h1) hi|-lo in M, - S on PE ----------
                f2s = Fd2
                for pp in range(2):
                    for e in range(2):
                        i = 4 * q + 2 * pp + e
                        for h in range(2):
                            f2s[(i, h)] = psp.tile([128, 512], F32, tag="F",
                                                   name=f"f2_{t}_{i}_{h}",
                                                   bufs=6)

                def h2_chain_list(pp, h, e):
                    p = 2 * q + pp
                    i = 4 * q + 2 * pp + e
                    fw = f2s[(i, h)][:, 0:NH]
                    key = ("h2", i, h)
                    mms = [(fw, idss[64 * e:64 * e + 64, :],
                            _ihalf(sH2[p], e, h),
                            True, False, (64 * e, 0), key)]
                    for k, (ky, kx) in enumerate(TAPS):
                        rhs = _cwin(sH1[p][64 * e:64 * e + 64, :], ky, kx,
                                    14 * h)
                        mms.append((fw, w3s[64 * e:64 * e + 64, k, :], rhs,
                                    False, k == 8, (64 * e, 0), key))
                    return mms

                waves = ([h2_chain_list(0, 0, 0), h2_chain_list(0, 1, 1),
                          h2_chain_list(1, 0, 0), h2_chain_list(1, 1, 1)],
                         [h2_chain_list(0, 0, 1), h2_chain_list(0, 1, 0),
                          h2_chain_list(1, 0, 1), h2_chain_list(1, 1, 0)])

                return waves

            def stage_h2_post(t, q):
                f2s = Fd2
                for pp in range(2):
                    p = 2 * q + pp
                    sP = scp.tile([128, NPIX], BF16, tag="s",
                                  name=f"sh2_{t}_{p}", bufs=8)
                    for e in range(2):
                        i = 4 * q + 2 * pp + e
                        lo = 64 * (1 - e)
                        hi = 64 * e
                        for h in range(2):
                            D = scp.tile([128, NH], F32, tag="d",
                                         name=f"d2_{t}_{i}_{h}", bufs=8)
                            nc.scalar.activation(
                                D[hi:hi + 64, :], f2s[(i, h)][lo:lo + 64, 0:NH],
                                ACTF.Identity, bias=tH2[hi:hi + 64, t:t + 1],
                                scale=1.0)
                            nc.vector.scalar_tensor_tensor(
                                sP[hi:hi + 64, NH * h:NH * h + NH],
                                f2s[(i, h)][hi:hi + 64, 0:NH],
                                0.0,
                                D[hi:hi + 64, :],
                                op0=ALU.add, op1=ALU.is_ge)
                    nc.vector.tensor_tensor(
                        _interior(sH2[p][:, :]), _interior(sH2[p][:, :]),
                        sP[:, :].rearrange("p (h w) -> p h w", w=28), ALU.add)

            def stage_h3_mm(t, q):
                # ---------- h3: 2x2 sum pool (x1.1 in compare) ----------
                f3s = Fd3
                for pp in range(2):
                    p = 2 * q + pp
                    f3s[p] = psp.tile([128, 512], F32, tag="F3",
                                      name=f"f3_{t}_{p}", bufs=2)

                def pool_chain_list(pp, e):
                    p = 2 * q + pp
                    col = 64 * e
                    fw = f3s[p][col:col + 64, 0:196]
                    key = ("h3", p)
                    mms = []
                    for wi, (dy, dx) in enumerate(
                            ((0, 0), (0, 1), (1, 0), (1, 1))):
                        rhs = _pwin(sH2[p][:, :], 64 * e, 64 * e + 64, dy, dx)
                        mms.append((fw, ids[64 * e:64 * e + 64, :], rhs,
                                    wi == 0, wi == 3, (64 * e, col), key))
                    return mms

                waves = ([pool_chain_list(0, 0), pool_chain_list(1, 1)],
                         [pool_chain_list(0, 1), pool_chain_list(1, 0)])

                return waves

            def stage_h3_post(t, q):
                f3s = Fd3
                for pp in range(2):
                    p = 2 * q + pp
                    sP = scp.tile([128, NPIX], BF16, tag="s",
                                  name=f"sh3_{t}_{p}", bufs=8)
                    nc.vector.scalar_tensor_tensor(
                        sP[:, 0:196], f3s[p][:, 0:196], 1.1,
                        qH3[p][:, :, t], op0=ALU.mult, op1=ALU.is_ge)
                    nc.vector.tensor_tensor(
                        qH3[p][:, :, t + 1], qH3[p][:, :, t],
                        sP[:, 0:196], ALU.add)

            # ================= time loop: L1(t+1) interleaves into t ====
            for q in range(qn):
                stage_l1(0, q)
            for t in range(nt):
                if t == 1:
                    for hl in range(2):
                        nc.sync.dma_start(
                            wf1l[:, hl, :, :],
                            _rap(wf1s, hl * 128 * 128,
                                 [[128, 128], [2 * 128 * 128, 98], [1, 128]]))
                for q in range(qn):
                    for wv in stage_h1_mm(t, q):
                        emit_wave(wv)
                    stage_h1_post(t, q)
                if t + 1 < nt:
                    for q in range(qn):
                        stage_l1(t + 1, q)
                for q in range(qn):
                    for wv in stage_h2_mm(t, q):
                        emit_wave(wv)
                    stage_h2_post(t, q)
                for q in range(qn):
                    for wv in stage_h3_mm(t, q):
                        emit_wave(wv)
                    stage_h3_post(t, q)

            # ---- ship Q trajectories to DRAM ----
            for p in range(pn):
                for e in range(2):
                    i = 2 * p + e
                    dst = _rap(sh3d, i * nt,
                               [[196 * ni * nt, 64], [ni * nt, 196], [1, nt]])
                    nc.sync.dma_start(dst, qH3[p][64 * e:64 * e + 64, :, 1:nt + 1])

            # ---------- fc1 ----------
            ff1 = psp.tile([128, 512], F32, tag="F3", name="ff1", bufs=2)
            CH = 14
            for c0 in range(0, 98, CH):
                rt = scp.tile([128, CH, ni * nt], BF16, tag="rf1t",
                              name=f"rf1t_{c0}", bufs=3)
                for j in range(2):
                    src = _rap(sh3d, (2 * c0 + j) * ni * nt,
                               [[196 * ni * nt, 64], [2 * ni * nt, CH],
                                [1, ni * nt]])
                    nc.sync.dma_start(rt[64 * j:64 * j + 64, :, :], src)
                for dx in range(CH):
                    px = c0 + dx
                    for hl in range(2):
                        st_ = (px == 0 and hl == 0)
                        sp_ = (px == 97 and hl == 1)
                        nc.tensor.matmul(ff1[0:64, 0:ni * nt],
                                         wf1l[:, hl, px, 0:64], rt[:, dx, :],
                                         start=st_, stop=sp_,
                                         tile_position=(0, 0),
                                         skip_group_check=True)
                        nc.tensor.matmul(ff1[64:128, 0:ni * nt],
                                         wf1l[:, hl, px, 64:128], rt[:, dx, :],
                                         start=st_, stop=sp_,
                                         tile_position=(0, 64),
                                         skip_group_check=True)

            # f1 spike scan (Q_h3 includes +1 offset; folded into thrF1)
            sf1 = stp.tile([128, ni], BF16, name="sf1")
            nc.gpsimd.memset(sf1[:, :], 0.0)
            for t in range(nt):
                sPf = scp.tile([128, ni], BF16, tag="sf", name=f"sf_{t}", bufs=2)
                nc.vector.scalar_tensor_tensor(
                    sPf[:, :],
                    ff1[:, 0:ni * nt].rearrange("p (n t) -> p n t", t=nt)[:, :, t],
                    tF1[:, t:t + 1], sf1[:, :],
                    op0=ALU.subtract, op1=ALU.is_ge)
                nc.vector.tensor_tensor(sf1[:, :], sf1[:, :], sPf[:, :], ALU.add)

            # ---------- fc2 + readout ----------
            ff2 = psp.tile([128, 512], F32, tag="F3", name="ff2", bufs=2)
            for hl in range(2):
                nc.tensor.matmul(ff2[0:10, 0:ni], wf2l[:, hl, :], sf1[:, :],
                                 start=(hl == 0), stop=(hl == 1))
            osb = scp.tile([10, ni], F32, tag="osb", name="osb")
            nc.scalar.activation(osb[:, :], ff2[0:10, 0:ni], ACTF.Identity,
                                 bias=bf2l[:, :], scale=1.0 / nt)
            nc.sync.dma_start(out[:, :].rearrange("n o -> o n"), osb[:, :])

    _split_excess_waits(nc)
    return nc


# ---------------- host side ----------------

def _split(a):
    hi = np.asarray(a, np.float64).astype(BF)
    lo = (np.asarray(a, np.float64) - hi.astype(np.float64)).astype(BF)
    return hi, lo


def _split3(a):
    a = np.asarray(a, np.float64)
    p0 = a.astype(BF)
    r = a - p0.astype(np.float64)
    p1 = r.astype(BF)
    p2 = (r - p1.astype(np.float64)).astype(BF)
    return p0, p1, p2


def _prep_shared(w1a, b1a, w1, b1, w2, b2, w3, b3, wf1, bf1, wf2, bf2, nt=T):
    d = {}
    # L1 lhsT [9, 128]: cols 0-63 w1a, 64-95 w1, 96-127 zero; rows k=3ky+kx
    # 3-term split W = W0+W1+W2; K=27 stacked variants to cover products with
    # the 3-term x split (X0,X1,X2): [W0;W1;W2], [W1;W0;0], [W2;0;W0]
    l1 = np.zeros((9, 128), np.float64)
    l1[:, 0:64] = w1a.reshape(64, 9).T
    l1[:, 64:96] = w1.reshape(32, 9).T
    W0, W1, W2 = _split3(l1)
    Z = np.zeros_like(W0)
    variants = [np.concatenate([W0, W1, W2]), np.concatenate([W1, W0, Z]),
                np.concatenate([W2, Z, W0])]
    w1qa = np.zeros((128, 3, 128), BF)
    for g in range(4):
        for v in range(3):
            w1qa[32 * g:32 * g + 27, v, :] = variants[v]
    d["w1q"] = w1qa
    # conv2 [tap, c, m]: hi at cols 0-63, -lo at cols 64-127, 4 row bases
    a2 = np.transpose(w2.reshape(64, 32, 3, 3), (2, 3, 1, 0)).reshape(9, 32, 64)
    h2v, l2v = _split(a2)
    w2ma = np.zeros((128, 9, 128), BF)
    for g in range(4):
        e = g % 2
        hi, lo = 64 * e, 64 * (1 - e)
        w2ma[32 * g:32 * g + 32, :, hi:hi + 64] = np.transpose(h2v, (1, 0, 2))
        w2ma[32 * g:32 * g + 32, :, lo:lo + 64] = -np.transpose(l2v, (1, 0, 2))
    d["w2m"] = w2ma
    a3 = np.transpose(w3.reshape(64, 64, 3, 3), (2, 3, 1, 0)).reshape(9, 64, 64)
    h3v, l3v = _split(a3)
    w3ma = np.zeros((128, 9, 128), BF)
    for e in range(2):
        hi, lo = 64 * e, 64 * (1 - e)
        w3ma[64 * e:64 * e + 64, :, hi:hi + 64] = np.transpose(h3v, (1, 0, 2))
        w3ma[64 * e:64 * e + 64, :, lo:lo + 64] = -np.transpose(l3v, (1, 0, 2))
    d["w3m"] = w3ma
    idqa = np.zeros((128, 64), BF)
    for e in range(2):
        idqa[64 * e:64 * e + 64, :] = np.eye(64, dtype=BF)
    d["idq"] = idqa
    idxa = np.zeros((128, 128), BF)
    idsa = np.zeros((128, 128), BF)
    for e in range(2):
        idxa[64 * e:64 * e + 64, 64 * e:64 * e + 64] = np.eye(64, dtype=BF)
        idsa[64 * e:64 * e + 64, 64 * e:64 * e + 64] = -np.eye(64, dtype=BF)
    d["idxq"] = idxa
    d["idsq"] = idsa
    # fc1 tiles [pxh, hl, row=64j+c, m]; fc1 consumes h3 spikes (no 1.1)
    wf1p = np.asarray(wf1, np.float64)                  # [128, 64, 14, 14]
    wf1f = wf1p.reshape(128, 64, 196)                   # px = 14*h + w
    wf1sa = np.zeros((98, 2, 128, 128), BF)
    hi1, lo1 = _split(wf1f)
    for pxh in range(98):
        for j in range(2):
            px = 2 * pxh + j
            wf1sa[pxh, 0, 64 * j:64 * j + 64, :] = hi1[:, :, px].T
            wf1sa[pxh, 1, 64 * j:64 * j + 64, :] = lo1[:, :, px].T
    d["wf1s"] = wf1sa
    h2v_, l2v_ = _split(np.asarray(wf2, np.float64).T)    # [128, 10]
    wf2sa = np.zeros((128, 2, 10), BF)
    wf2sa[:, 0, :] = h2v_
    wf2sa[:, 1, :] = l2v_
    d["wf2s"] = wf2sa
    # thresholds [128, nt]
    tsteps = np.arange(1, nt + 1)
    thrL1 = np.ones((128, nt), np.float32)
    thrL1[0:64] = 1.0 - tsteps[None, :] * np.asarray(b1a, np.float64)[:, None]
    thrL1[64:96] = 1.0 - tsteps[None, :] * np.asarray(b1, np.float64)[:, None]
    d["thrL1"] = thrL1
    thrH1 = np.ones((128, nt), np.float32)
    b2d = np.asarray(b2, np.float64)
    thrH1[0:64] = 1.0 - tsteps[None, :] * b2d[:, None]
    thrH1[64:128] = 1.0 - tsteps[None, :] * b2d[:, None]
    d["thrH1"] = thrH1
    thrH2 = np.ones((128, nt), np.float32)
    b3d = np.asarray(b3, np.float64)
    thrH2[0:64] = 1.0 - tsteps[None, :] * b3d[:, None]
    thrH2[64:128] = 1.0 - tsteps[None, :] * b3d[:, None]
    d["thrH2"] = thrH2
    # f1: (F_comp = wf1p . Q) >= S + 1 + rowsum(wf1p) - (t+1)*bf1
    rs = wf1f.sum(axis=(1, 2))                          # [128]
    thrF1 = np.zeros((128, nt), np.float32)
    thrF1[:, :] = (1.0 + rs[:, None]
                   - tsteps[None, :] * np.asarray(bf1, np.float64)[:, None])
    d["thrF1"] = thrF1
    d["bf2t"] = np.asarray(bf2, np.float32).reshape(10, 1)
    return d


def _prep_x(xc, nt=T):
    """xc [ni, 1, 28, 28, nt] -> host im2col of the 3-term-split cumulative
    input, stacked [ni*nt, 27, 2, 392] = ([X0;X1;X2], half, px)."""
    ni = xc.shape[0]
    X = np.cumsum(np.asarray(xc, np.float64), axis=-1)[:, 0]   # [ni, 28, 28, nt]
    X = np.moveaxis(X, -1, 1)                                  # [ni, nt, 28, 28]
    pad = np.zeros((ni * nt, 30, 30), np.float64)
    pad[:, 1:29, 1:29] = X.reshape(ni * nt, 28, 28)
    col = np.zeros((ni * nt, 9, 2, NH), np.float64)
    for k, (ky, kx) in enumerate(TAPS):
        for h in range(2):
            col[:, k, h, :] = pad[:, 14 * h + ky:14 * h + ky + 14,
                                  kx:kx + 28].reshape(ni * nt, NH)
    x0, x1, x2 = _split3(col)
    return np.concatenate([x0, x1, x2], axis=1)  # [ni*nt, 27, 2, NH]


_NC_CACHE = {}


def kernel(x, w1a, b1a, w1, b1, w2, b2, w3, b3, wf1, bf1, wf2, bf2):
    x = np.asarray(x)
    n_total = x.shape[0]
    ni = n_total // NCORE
    key = (ni, T)
    if key not in _NC_CACHE:
        _NC_CACHE[key] = build_nc(T, ni)
    nc = _NC_CACHE[key]

    shared = _prep_shared(w1a, b1a, w1, b1, w2, b2, w3, b3, wf1, bf1, wf2, bf2)
    in_maps = []
    for c in range(NCORE):
        m = dict(shared)
        m["xst"] = _prep_x(x[c * ni:(c + 1) * ni])
        in_maps.append(m)

    res = run_bass_kernel_spmd(nc, in_maps, list(range(NCORE))).results
    return np.concatenate([res[c]["out"] for c in range(NCORE)], axis=0)


# revision 4
# speedup vs baseline: 1.0648x; 1.0300x over previous
"""CatNet spiking CNN on 8x TRN2 NeuronCores (data-parallel over batch N=64).

v2: conv2/conv3 stream ONE column pass per tap with the bf16 hi/lo weight
split packed into the M dimension: lhs [K, 128] has W_hi at out-cols 0-63 and
-W_lo at cols 64-127, so PSUM partitions 0-63 hold (Whi.S + X - S_own) and
64-127 hold (-Wlo.S). X-inject and the state subtract ride the PE as identity
matmuls. The spike decision is then one DVE op per (image, half):
    s = is_ge(psum_hi - thr, psum_neg_lo)   <=>  hi+lo+X-S >= thr.
This halves conv2/conv3 tensor columns vs the 2-pass hi/lo while keeping the
exact-product precision class. States are small integers -> exact in bf16.
"""
import numpy as np
import ml_dtypes

import concourse.bass as bass
import concourse.mybir as mybir
from concourse import tile
from concourse.bass_utils import run_bass_kernel_spmd
from concourse.tile_rust import add_dep_helper

F32 = mybir.dt.float32
BF16 = mybir.dt.bfloat16
BF = ml_dtypes.bfloat16
ALU = mybir.AluOpType
ACTF = mybir.ActivationFunctionType

NCORE = 8
NI = 8
T = 16
WP = 32                 # padded row stride (30 cols used; 64B-aligned bf16 rows)
HPAD = 30
PADPIX = HPAD * WP      # 960
NPIX = 784
NH = 392
TAPS = [(ky, kx) for ky in range(3) for kx in range(3)]


def _split_excess_waits(nc, maxw=1):
    """This walrus accepts only one sync-wait per instruction; hoist extras
    onto preceding same-engine nops."""
    ctr = [0]
    for bb in nc.m.functions[0].blocks:
        nl = []
        for inst in bb.instructions:
            w = list(inst.sync_info.on_wait) if inst.sync_info else []
            if len(w) > maxw:
                keep, exc = w[:maxw], w[maxw:]
                for cs in range(0, len(exc), maxw):
                    nop = mybir.InstNoOp(name=f"I-ws-{ctr[0]}", ins=[], outs=[])
                    ctr[0] += 1
                    nop.engine = inst.engine
                    nop.sync_info = mybir.SyncInfo(
                        on_wait=list(exc[cs:cs + maxw]), on_update=[])
                    nc.register_instruction(nop)
                    nl.append(nop)
                inst.sync_info = mybir.SyncInfo(
                    on_wait=list(keep), on_update=list(inst.sync_info.on_update))
            nl.append(inst)
        bb.instructions[:] = nl


def _rap(handle, offset, dims):
    """Raw access pattern on a DRAM tensor handle."""
    return bass.AP(handle, offset, [list(d) for d in dims])


def _pad3(ap_pad, nrow=128):
    """[P, 960] padded tile -> [P, 30, 32] view."""
    return ap_pad.rearrange("p (h w) -> p h w", w=WP)


def _interior(ap_pad):
    """[P, 960] padded tile -> [P, 28, 28] interior view."""
    return _pad3(ap_pad)[:, 1:29, 1:29]


def _cwin(ap_pad, ky, kx, h0):
    """Conv-tap window: out rows h0..h0+13 -> padded rows h0+ky.., cols kx.."""
    return _pad3(ap_pad)[:, h0 + ky:h0 + ky + 14, kx:kx + 28]


def _ihalf(ap_pad, e, h):
    """Interior half view [64, 14, 28] of image e in a padded pair tile."""
    v = ap_pad[64 * e:64 * e + 64, :].rearrange("p (h w) -> p h w", w=WP)
    return v[:, 1 + 14 * h:15 + 14 * h, 1:29]


def _pwin(ap_pad, e0, e1, dy, dx):
    """Pool window on partitions e0:e1: [P, 14, 14], elem (r,c) = padded
    (2r+1+dy, 2c+1+dx)."""
    v = ap_pad[e0:e1, :].rearrange("p (h w) -> p h w", w=WP)
    return v[:, 1 + dy:1 + dy + 28:2, 1 + dx:1 + dx + 28:2]


def build_nc(nt=T, ni=NI):
    nc = bass.Bass()
    qn = ni // 4
    pn = ni // 2

    xst = nc.dram_tensor("xst", [ni * nt, 27, 2, NH], BF16, kind="ExternalInput")
    w1q = nc.dram_tensor("w1q", [128, 3, 128], BF16, kind="ExternalInput")
    w2m = nc.dram_tensor("w2m", [128, 9, 128], BF16, kind="ExternalInput")
    w3m = nc.dram_tensor("w3m", [128, 9, 128], BF16, kind="ExternalInput")
    idq = nc.dram_tensor("idq", [128, 64], BF16, kind="ExternalInput")
    idxq = nc.dram_tensor("idxq", [128, 128], BF16, kind="ExternalInput")
    idsq = nc.dram_tensor("idsq", [128, 128], BF16, kind="ExternalInput")
    wf1s = nc.dram_tensor("wf1s", [98, 2, 128, 128], BF16, kind="ExternalInput")
    wf2s = nc.dram_tensor("wf2s", [128, 2, 10], BF16, kind="ExternalInput")
    thrL1 = nc.dram_tensor("thrL1", [128, nt], F32, kind="ExternalInput")
    thrH1 = nc.dram_tensor("thrH1", [128, nt], F32, kind="ExternalInput")
    thrH2 = nc.dram_tensor("thrH2", [128, nt], F32, kind="ExternalInput")
    thrF1 = nc.dram_tensor("thrF1", [128, nt], F32, kind="ExternalInput")
    bf2t = nc.dram_tensor("bf2t", [10, 1], F32, kind="ExternalInput")
    out = nc.dram_tensor("out", [ni, 10], F32, kind="ExternalOutput")
    sh3d = nc.dram_tensor("sh3d", [64, 196, ni, nt], BF16, kind="Internal")

    with tile.TileContext(nc) as tc:
        with (
            tc.tile_pool(name="wpool", bufs=1) as wp,
            tc.tile_pool(name="state", bufs=1) as stp,
            tc.tile_pool(name="scratch", bufs=1) as scp,
            tc.tile_pool(name="psum", bufs=1, space="PSUM") as psp,
        ):
            # ---- weights / thresholds ----
            w1s = wp.tile([128, 3, 128], BF16, name="w1s")
            nc.sync.dma_start(w1s[:, :, :], w1q[:, :, :])
            w2s = wp.tile([128, 9, 128], BF16, name="w2s")
            nc.sync.dma_start(w2s[:, :, :], w2m[:, :, :])
            w3s = wp.tile([128, 9, 128], BF16, name="w3s")
            nc.sync.dma_start(w3s[:, :, :], w3m[:, :, :])
            ids = wp.tile([128, 64], BF16, name="ids")
            nc.sync.dma_start(ids[:, :], idq[:, :])
            idxs = wp.tile([128, 128], BF16, name="idxs")
            nc.sync.dma_start(idxs[:, :], idxq[:, :])
            idss = wp.tile([128, 128], BF16, name="idss")
            nc.sync.dma_start(idss[:, :], idsq[:, :])
            wf2l = wp.tile([128, 2, 10], BF16, name="wf2l")
            nc.sync.dma_start(wf2l[:, :, :], wf2s[:, :, :])
            tL1 = wp.tile([128, nt], F32, name="tL1")
            nc.sync.dma_start(tL1[:, :], thrL1[:, :])
            tH1 = wp.tile([128, nt], F32, name="tH1")
            nc.sync.dma_start(tH1[:, :], thrH1[:, :])
            tH2 = wp.tile([128, nt], F32, name="tH2")
            nc.sync.dma_start(tH2[:, :], thrH2[:, :])
            tF1 = wp.tile([128, nt], F32, name="tF1")
            nc.sync.dma_start(tF1[:, :], thrF1[:, :])
            bf2l = wp.tile([10, 1], F32, name="bf2l")
            nc.sync.dma_start(bf2l[:, :], bf2t[:, :])
            wf1l = wp.tile([128, 2, 98, 128], BF16, name="wf1l")

            # ---- persistent states ----
            sL1 = []        # per image [128, 784]: rows 0-63 S_x1, 64-95 S_h0
            for i in range(ni):
                st = stp.tile([128, NPIX], BF16, name=f"sL1_{i}")
                nc.gpsimd.memset(st[:, :], 0.0)
                sL1.append(st)
            sH0q = []       # per quad, padded, img g at rows 32g (conv2 rhs)
            for q in range(qn):
                st = stp.tile([128, PADPIX], BF16, name=f"sH0q_{q}")
                nc.gpsimd.memset(st[:, :], 0.0)
                sH0q.append(st)
            sX1 = []        # per pair, padded, img e at rows 64e (x1 spikes)
            for p in range(pn):
                st = stp.tile([128, PADPIX], BF16, name=f"sX1_{p}")
                nc.gpsimd.memset(st[:, :], 0.0)
                sX1.append(st)
            sH1, sH2, qH3 = [], [], []
            for p in range(pn):
                s1 = stp.tile([128, PADPIX], BF16, name=f"sH1_{p}")
                nc.gpsimd.memset(s1[:, :], 0.0)
                sH1.append(s1)
                s2 = stp.tile([128, PADPIX], BF16, name=f"sH2_{p}")
                nc.gpsimd.memset(s2[:, :], 0.0)
                sH2.append(s2)
                q3 = stp.tile([128, 196, nt + 1], BF16, name=f"qH3_{p}")
                nc.gpsimd.memset(q3[:, :, :], 1.0)   # Q = S + 1
                qH3.append(q3)

            last_in_bank = {}  # chain-key -> last matmul inst of prior chain

            def chain_dep(key, first_mm):
                if key in last_in_bank:
                    add_dep_helper(first_mm.ins, last_in_bank[key].ins,
                                   sync=False, reason="psum group order")

            FdL = {}
            Fd1 = {}
            Fd2 = {}
            Fd3 = {}

            def emit_wave(chains):
                n = max(len(c) for c in chains)
                for j in range(n):
                    for c in chains:
                        if j < len(c):
                            fw, lhs, rhs, st, sp, tp, key = c[j]
                            mm = nc.tensor.matmul(fw, lhs, rhs, start=st,
                                                  stop=sp, tile_position=tp)
                            if st:
                                chain_dep(key, mm)
                            if sp:
                                last_in_bank[key] = mm

            def stage_l1(t, q):
                imgs = [4 * q + g for g in range(4)]
                # ---------- L1 ----------
                imst = scp.tile([128, NPIX], BF16, tag="imst",
                                name=f"imst_{t}_{q}", bufs=3)
                for g, i in enumerate(imgs):
                    off = (i * nt + t) * 27 * 2 * NH
                    nc.sync.dma_start(imst[32 * g:32 * g + 27, :],
                                      _rap(xst, off, [[2 * NH, 27], [1, 2 * NH]]))
                fLs = FdL
                l1_mms = {i: [] for i in imgs}
                for g, i in enumerate(imgs):
                    for h in range(2):
                        fLs[(i, h)] = psp.tile([128, 512], F32, tag="F",
                                               name=f"fL_{t}_{i}_{h}", bufs=7)
                        fw = fLs[(i, h)][:, 0:NH]
                        rh = imst[32 * g:32 * g + 27, NH * h:NH * h + NH]
                        for v in range(3):
                            l1_mms[i].append(
                                (fw, w1s[32 * g:32 * g + 27, v, :], rh,
                                 v == 0, v == 2, (32 * g, 0), ("L", i, h)))
                # round-robin across images for row-group diversity
                for j in range(6):
                    for g, i in enumerate(imgs):
                        fw, lhs, rh, st, sp, tp, key = l1_mms[i][j]
                        mm = nc.tensor.matmul(fw, lhs, rh, start=st, stop=sp,
                                              tile_position=tp)
                        if st:
                            chain_dep(key, mm)
                        if sp:
                            last_in_bank[key] = mm
                for g, i in enumerate(imgs):
                    sL = scp.tile([128, NPIX], BF16, tag="s",
                                  name=f"s_{t}_{i}", bufs=8)
                    for h in range(2):
                        nc.vector.scalar_tensor_tensor(
                            sL[:, NH * h:NH * h + NH],
                            fLs[(i, h)][:, 0:NH],
                            tL1[:, t:t + 1],
                            sL1[i][:, NH * h:NH * h + NH],
                            op0=ALU.subtract, op1=ALU.is_ge)
                    nc.vector.tensor_tensor(
                        sL1[i][:, :], sL1[i][:, :], sL[:, :], ALU.add)
                    # maintain padded quad h0 copy on ScalarE (partition shift)
                    nc.scalar.copy(
                        _interior(sH0q[q][32 * g:32 * g + 32, :]),
                        sL1[i][64:96, :].rearrange("p (h w) -> p h w", w=28))
                    # maintain padded x1 pair copy (img e at rows 64e)
                    p_ = i // 2
                    e_ = i % 2
                    nc.scalar.copy(
                        _interior(sX1[p_][64 * e_:64 * e_ + 64, :]),
                        sL1[i][0:64, :].rearrange("p (h w) -> p h w", w=28))

            def stage_h1_mm(t, q):
                # ---------- h1: conv2(S_h0) hi|-lo in M + X - S on PE ----------
                f1s = Fd1
                for pp in range(2):
                    for e in range(2):
                        i = 4 * q + 2 * pp + e
                        for h in range(2):
                            f1s[(i, h)] = psp.tile([128, 512], F32, tag="F",
                                                   name=f"f1_{t}_{i}_{h}",
                                                   bufs=7)

                def h1_chain_list(pp, h, e):
                    p = 2 * q + pp
                    i = 4 * q + 2 * pp + e
                    g = 2 * pp + e
                    fw = f1s[(i, h)][:, 0:NH]
                    key = ("h1", i, h)
                    mms = [
                        (fw, idxs[64 * e:64 * e + 64, :], _ihalf(sX1[p], e, h),
                         True, False, (64 * e, 0), key),
                        (fw, idss[64 * e:64 * e + 64, :], _ihalf(sH1[p], e, h),
                         False, False, (64 * e, 0), key),
                    ]
                    for k, (ky, kx) in enumerate(TAPS):
                        rhs = _cwin(sH0q[q][32 * g:32 * g + 32, :], ky, kx,
                                    14 * h)
                        mms.append((fw, w2s[32 * g:32 * g + 32, k, :], rhs,
                                    False, k == 8, (32 * g, 0), key))
                    return mms

                waves = ([h1_chain_list(0, 0, 0), h1_chain_list(0, 1, 1),
                          h1_chain_list(1, 0, 0), h1_chain_list(1, 1, 1)],
                         [h1_chain_list(0, 0, 1), h1_chain_list(0, 1, 0),
                          h1_chain_list(1, 0, 1), h1_chain_list(1, 1, 0)])

                return waves

            def stage_h1_post(t, q):
                f1s = Fd1
                for pp in range(2):
                    p = 2 * q + pp
                    sP = scp.tile([128, NPIX], BF16, tag="s",
                                  name=f"sh1_{t}_{p}", bufs=8)
                    for e in range(2):
                        i = 4 * q + 2 * pp + e
                        lo = 64 * (1 - e)
                        hi = 64 * e
                        for h in range(2):
                            D = scp.tile([128, NH], F32, tag="d",
                                         name=f"d1_{t}_{i}_{h}", bufs=8)
                            nc.scalar.activation(
                                D[hi:hi + 64, :], f1s[(i, h)][lo:lo + 64, 0:NH],
                                ACTF.Identity, bias=tH1[hi:hi + 64, t:t + 1],
                                scale=1.0)
                            nc.vector.scalar_tensor_tensor(
                                sP[hi:hi + 64, NH * h:NH * h + NH],
                                f1s[(i, h)][hi:hi + 64, 0:NH],
                                0.0,
                                D[hi:hi + 64, :],
                                op0=ALU.add, op1=ALU.is_ge)
                    nc.vector.tensor_tensor(
                        _interior(sH1[p][:, :]), _interior(sH1[p][:, :]),
                        sP[:, :].rearrange("p (h w) -> p h w", w=28), ALU.add)

            def stage_h2_mm(t, q):
                # ---------- h2: conv3(S_h1) hi|-lo in M, - S on PE ----------
                f2s = Fd2
                for pp in range(2):
                    for e in range(2):
                        i = 4 * q + 2 * pp + e
                        for h in range(2):
                            f2s[(i, h)] = psp.tile([128, 512], F32, tag="F",
                                                   name=f"f2_{t}_{i}_{h}",
                                                   bufs=7)

                def h2_chain_list(pp, h, e):
                    p = 2 * q + pp
                    i = 4 * q + 2 * pp + e
                    fw = f2s[(i, h)][:, 0:NH]
                    key = ("h2", i, h)
                    mms = [(fw, idss[64 * e:64 * e + 64, :],
                            _ihalf(sH2[p], e, h),
                            True, False, (64 * e, 0), key)]
                    for k, (ky, kx) in enumerate(TAPS):
                        rhs = _cwin(sH1[p][64 * e:64 * e + 64, :], ky, kx,
                                    14 * h)
                        mms.append((fw, w3s[64 * e:64 * e + 64, k, :], rhs,
                                    False, k == 8, (64 * e, 0), key))
                    return mms

                waves = ([h2_chain_list(0, 0, 0), h2_chain_list(0, 1, 1),
                          h2_chain_list(1, 0, 0), h2_chain_list(1, 1, 1)],
                         [h2_chain_list(0, 0, 1), h2_chain_list(0, 1, 0),
                          h2_chain_list(1, 0, 1), h2_chain_list(1, 1, 0)])

                return waves

            def stage_h2_post(t, q):
                f2s = Fd2
                for pp in range(2):
                    p = 2 * q + pp
                    sP = scp.tile([128, NPIX], BF16, tag="s",
                                  name=f"sh2_{t}_{p}", bufs=8)
                    for e in range(2):
                        i = 4 * q + 2 * pp + e
                        lo = 64 * (1 - e)
                        hi = 64 * e
                        for h in range(2):
                            D = scp.tile([128, NH], F32, tag="d",
                                         name=f"d2_{t}_{i}_{h}", bufs=8)
                            nc.scalar.activation(
                                D[hi:hi + 64, :], f2s[(i, h)][lo:lo + 64, 0:NH],
                                ACTF.Identity, bias=tH2[hi:hi + 64, t:t + 1],
                                scale=1.0)
                            nc.vector.scalar_tensor_tensor(
                                sP[hi:hi + 64, NH * h:NH * h + NH],
                                f2s[(i, h)][hi:hi + 64, 0:NH],
                                0.0,
                                D[hi:hi + 64, :],
                                op0=ALU.add, op1=ALU.is_ge)
                    nc.vector.tensor_tensor(
                        _interior(sH2[p][:, :]), _interior(sH2[p][:, :]),
                        sP[:, :].rearrange("p (h w) -> p h w", w=28), ALU.add)

            def stage_h3(t, q):
                # ---------- h3: 2x2 sum pool on DVE (sums <=64, bf16-exact;
                # x1.1 applied in the compare, identical decisions) ----------
                for pp in range(2):
                    p = 2 * q + pp
                    v = _pad3(sH2[p][:, :])
                    pooled = scp.tile([128, 196], BF16, tag="pool",
                                      name=f"pl_{t}_{p}", bufs=4)
                    pv = pooled[:, :].rearrange("p (a b) -> p a b", b=14)
                    nc.vector.tensor_tensor(
                        pv, v[:, 1:29:2, 1:29:2], v[:, 1:29:2, 2:30:2],
                        ALU.add)
                    nc.vector.tensor_tensor(
                        pv, pv, v[:, 2:30:2, 1:29:2], ALU.add)
                    nc.vector.tensor_tensor(
                        pv, pv, v[:, 2:30:2, 2:30:2], ALU.add)
                    sP = scp.tile([128, NPIX], BF16, tag="s",
                                  name=f"sh3_{t}_{p}", bufs=8)
                    nc.vector.scalar_tensor_tensor(
                        sP[:, 0:196], pooled[:, :], 1.1,
                        qH3[p][:, :, t], op0=ALU.mult, op1=ALU.is_ge)
                    nc.vector.tensor_tensor(
                        qH3[p][:, :, t + 1], qH3[p][:, :, t],
                        sP[:, 0:196], ALU.add)

            # ================= time loop: L1(t+1) interleaves into t ====
            for q in range(qn):
                stage_l1(0, q)
            for t in range(nt):
                if t == 1:
                    for hl in range(2):
                        nc.sync.dma_start(
                            wf1l[:, hl, :, :],
                            _rap(wf1s, hl * 128 * 128,
                                 [[128, 128], [2 * 128 * 128, 98], [1, 128]]))
                for q in range(qn):
                    for wv in stage_h1_mm(t, q):
                        emit_wave(wv)
                    stage_h1_post(t, q)
                if t + 1 < nt:
                    for q in range(qn):
                        stage_l1(t + 1, q)
                for q in range(qn):
                    for wv in stage_h2_mm(t, q):
                        emit_wave(wv)
                    stage_h2_post(t, q)
                for q in range(qn):
                    stage_h3(t, q)

            # ---- ship Q trajectories to DRAM ----
            for p in range(pn):
                for e in range(2):
                    i = 2 * p + e
                    dst = _rap(sh3d, i * nt,
                               [[196 * ni * nt, 64], [ni * nt, 196], [1, nt]])
                    nc.sync.dma_start(dst, qH3[p][64 * e:64 * e + 64, :, 1:nt + 1])

            # ---------- fc1 ----------
            ff1 = psp.tile([128, 512], F32, tag="F3", name="ff1", bufs=1)
            CH = 14
            for c0 in range(0, 98, CH):
                rt = scp.tile([128, CH, ni * nt], BF16, tag="rf1t",
                              name=f"rf1t_{c0}", bufs=3)
                for j in range(2):
                    src = _rap(sh3d, (2 * c0 + j) * ni * nt,
                               [[196 * ni * nt, 64], [2 * ni * nt, CH],
                                [1, ni * nt]])
                    nc.sync.dma_start(rt[64 * j:64 * j + 64, :, :], src)
                for dx in range(CH):
                    px = c0 + dx
                    for hl in range(2):
                        st_ = (px == 0 and hl == 0)
                        sp_ = (px == 97 and hl == 1)
                        nc.tensor.matmul(ff1[0:64, 0:ni * nt],
                                         wf1l[:, hl, px, 0:64], rt[:, dx, :],
                                         start=st_, stop=sp_,
                                         tile_position=(0, 0),
                                         skip_group_check=True)
                        nc.tensor.matmul(ff1[64:128, 0:ni * nt],
                                         wf1l[:, hl, px, 64:128], rt[:, dx, :],
                                         start=st_, stop=sp_,
                                         tile_position=(0, 64),
                                         skip_group_check=True)

            # f1 spike scan (Q_h3 includes +1 offset; folded into thrF1)
            sf1 = stp.tile([128, ni], BF16, name="sf1")
            nc.gpsimd.memset(sf1[:, :], 0.0)
            for t in range(nt):
                sPf = scp.tile([128, ni], BF16, tag="sf", name=f"sf_{t}", bufs=2)
                nc.vector.scalar_tensor_tensor(
                    sPf[:, :],
                    ff1[:, 0:ni * nt].rearrange("p (n t) -> p n t", t=nt)[:, :, t],
                    tF1[:, t:t + 1], sf1[:, :],
                    op0=ALU.subtract, op1=ALU.is_ge)
                nc.vector.tensor_tensor(sf1[:, :], sf1[:, :], sPf[:, :], ALU.add)

            # ---------- fc2 + readout ----------
            ff2 = psp.tile([128, 512], F32, tag="F3", name="ff2", bufs=1)
            for hl in range(2):
                nc.tensor.matmul(ff2[0:10, 0:ni], wf2l[:, hl, :], sf1[:, :],
                                 start=(hl == 0), stop=(hl == 1))
            osb = scp.tile([10, ni], F32, tag="osb", name="osb")
            nc.scalar.activation(osb[:, :], ff2[0:10, 0:ni], ACTF.Identity,
                                 bias=bf2l[:, :], scale=1.0 / nt)
            nc.sync.dma_start(out[:, :].rearrange("n o -> o n"), osb[:, :])

    _split_excess_waits(nc)
    return nc


# ---------------- host side ----------------

def _split(a):
    hi = np.asarray(a, np.float64).astype(BF)
    lo = (np.asarray(a, np.float64) - hi.astype(np.float64)).astype(BF)
    return hi, lo


def _split3(a):
    a = np.asarray(a, np.float64)
    p0 = a.astype(BF)
    r = a - p0.astype(np.float64)
    p1 = r.astype(BF)
    p2 = (r - p1.astype(np.float64)).astype(BF)
    return p0, p1, p2


def _prep_shared(w1a, b1a, w1, b1, w2, b2, w3, b3, wf1, bf1, wf2, bf2, nt=T):
    d = {}
    # L1 lhsT [9, 128]: cols 0-63 w1a, 64-95 w1, 96-127 zero; rows k=3ky+kx
    # 3-term split W = W0+W1+W2; K=27 stacked variants to cover products with
    # the 3-term x split (X0,X1,X2): [W0;W1;W2], [W1;W0;0], [W2;0;W0]
    l1 = np.zeros((9, 128), np.float64)
    l1[:, 0:64] = w1a.reshape(64, 9).T
    l1[:, 64:96] = w1.reshape(32, 9).T
    W0, W1, W2 = _split3(l1)
    Z = np.zeros_like(W0)
    variants = [np.concatenate([W0, W1, W2]), np.concatenate([W1, W0, Z]),
                np.concatenate([W2, Z, W0])]
    w1qa = np.zeros((128, 3, 128), BF)
    for g in range(4):
        for v in range(3):
            w1qa[32 * g:32 * g + 27, v, :] = variants[v]
    d["w1q"] = w1qa
    # conv2 [tap, c, m]: hi at cols 0-63, -lo at cols 64-127, 4 row bases
    a2 = np.transpose(w2.reshape(64, 32, 3, 3), (2, 3, 1, 0)).reshape(9, 32, 64)
    h2v, l2v = _split(a2)
    w2ma = np.zeros((128, 9, 128), BF)
    for g in range(4):
        e = g % 2
        hi, lo = 64 * e, 64 * (1 - e)
        w2ma[32 * g:32 * g + 32, :, hi:hi + 64] = np.transpose(h2v, (1, 0, 2))
        w2ma[32 * g:32 * g + 32, :, lo:lo + 64] = -np.transpose(l2v, (1, 0, 2))
    d["w2m"] = w2ma
    a3 = np.transpose(w3.reshape(64, 64, 3, 3), (2, 3, 1, 0)).reshape(9, 64, 64)
    h3v, l3v = _split(a3)
    w3ma = np.zeros((128, 9, 128), BF)
    for e in range(2):
        hi, lo = 64 * e, 64 * (1 - e)
        w3ma[64 * e:64 * e + 64, :, hi:hi + 64] = np.transpose(h3v, (1, 0, 2))
        w3ma[64 * e:64 * e + 64, :, lo:lo + 64] = -np.transpose(l3v, (1, 0, 2))
    d["w3m"] = w3ma
    idqa = np.zeros((128, 64), BF)
    for e in range(2):
        idqa[64 * e:64 * e + 64, :] = np.eye(64, dtype=BF)
    d["idq"] = idqa
    idxa = np.zeros((128, 128), BF)
    idsa = np.zeros((128, 128), BF)
    for e in range(2):
        idxa[64 * e:64 * e + 64, 64 * e:64 * e + 64] = np.eye(64, dtype=BF)
        idsa[64 * e:64 * e + 64, 64 * e:64 * e + 64] = -np.eye(64, dtype=BF)
    d["idxq"] = idxa
    d["idsq"] = idsa
    # fc1 tiles [pxh, hl, row=64j+c, m]; fc1 consumes h3 spikes (no 1.1)
    wf1p = np.asarray(wf1, np.float64)                  # [128, 64, 14, 14]
    wf1f = wf1p.reshape(128, 64, 196)                   # px = 14*h + w
    wf1sa = np.zeros((98, 2, 128, 128), BF)
    hi1, lo1 = _split(wf1f)
    for pxh in range(98):
        for j in range(2):
            px = 2 * pxh + j
            wf1sa[pxh, 0, 64 * j:64 * j + 64, :] = hi1[:, :, px].T
            wf1sa[pxh, 1, 64 * j:64 * j + 64, :] = lo1[:, :, px].T
    d["wf1s"] = wf1sa
    h2v_, l2v_ = _split(np.asarray(wf2, np.float64).T)    # [128, 10]
    wf2sa = np.zeros((128, 2, 10), BF)
    wf2sa[:, 0, :] = h2v_
    wf2sa[:, 1, :] = l2v_
    d["wf2s"] = wf2sa
    # thresholds [128, nt]
    tsteps = np.arange(1, nt + 1)
    thrL1 = np.ones((128, nt), np.float32)
    thrL1[0:64] = 1.0 - tsteps[None, :] * np.asarray(b1a, np.float64)[:, None]
    thrL1[64:96] = 1.0 - tsteps[None, :] * np.asarray(b1, np.float64)[:, None]
    d["thrL1"] = thrL1
    thrH1 = np.ones((128, nt), np.float32)
    b2d = np.asarray(b2, np.float64)
    thrH1[0:64] = 1.0 - tsteps[None, :] * b2d[:, None]
    thrH1[64:128] = 1.0 - tsteps[None, :] * b2d[:, None]
    d["thrH1"] = thrH1
    thrH2 = np.ones((128, nt), np.float32)
    b3d = np.asarray(b3, np.float64)
    thrH2[0:64] = 1.0 - tsteps[None, :] * b3d[:, None]
    thrH2[64:128] = 1.0 - tsteps[None, :] * b3d[:, None]
    d["thrH2"] = thrH2
    # f1: (F_comp = wf1p . Q) >= S + 1 + rowsum(wf1p) - (t+1)*bf1
    rs = wf1f.sum(axis=(1, 2))                          # [128]
    thrF1 = np.zeros((128, nt), np.float32)
    thrF1[:, :] = (1.0 + rs[:, None]
                   - tsteps[None, :] * np.asarray(bf1, np.float64)[:, None])
    d["thrF1"] = thrF1
    d["bf2t"] = np.asarray(bf2, np.float32).reshape(10, 1)
    return d


def _prep_x(xc, nt=T):
    """xc [ni, 1, 28, 28, nt] -> host im2col of the 3-term-split cumulative
    input, stacked [ni*nt, 27, 2, 392] = ([X0;X1;X2], half, px)."""
    ni = xc.shape[0]
    X = np.cumsum(np.asarray(xc, np.float64), axis=-1)[:, 0]   # [ni, 28, 28, nt]
    X = np.moveaxis(X, -1, 1)                                  # [ni, nt, 28, 28]
    pad = np.zeros((ni * nt, 30, 30), np.float64)
    pad[:, 1:29, 1:29] = X.reshape(ni * nt, 28, 28)
    col = np.zeros((ni * nt, 9, 2, NH), np.float64)
    for k, (ky, kx) in enumerate(TAPS):
        for h in range(2):
            col[:, k, h, :] = pad[:, 14 * h + ky:14 * h + ky + 14,
                                  kx:kx + 28].reshape(ni * nt, NH)
    x0, x1, x2 = _split3(col)
    return np.concatenate([x0, x1, x2], axis=1)  # [ni*nt, 27, 2, NH]


_NC_CACHE = {}


def kernel(x, w1a, b1a, w1, b1, w2, b2, w3, b3, wf1, bf1, wf2, bf2):
    x = np.asarray(x)
    n_total = x.shape[0]
    ni = n_total // NCORE
    key = (ni, T)
    if key not in _NC_CACHE:
        _NC_CACHE[key] = build_nc(T, ni)
    nc = _NC_CACHE[key]

    shared = _prep_shared(w1a, b1a, w1, b1, w2, b2, w3, b3, wf1, bf1, wf2, bf2)
    in_maps = []
    for c in range(NCORE):
        m = dict(shared)
        m["xst"] = _prep_x(x[c * ni:(c + 1) * ni])
        in_maps.append(m)

    res = run_bass_kernel_spmd(nc, in_maps, list(range(NCORE))).results
    return np.concatenate([res[c]["out"] for c in range(NCORE)], axis=0)


# revision 5
# speedup vs baseline: 1.0704x; 1.0052x over previous
"""CatNet spiking CNN on 8x TRN2 NeuronCores (data-parallel over batch N=64).

v2: conv2/conv3 stream ONE column pass per tap with the bf16 hi/lo weight
split packed into the M dimension: lhs [K, 128] has W_hi at out-cols 0-63 and
-W_lo at cols 64-127, so PSUM partitions 0-63 hold (Whi.S + X - S_own) and
64-127 hold (-Wlo.S). X-inject and the state subtract ride the PE as identity
matmuls. The spike decision is then one DVE op per (image, half):
    s = is_ge(psum_hi - thr, psum_neg_lo)   <=>  hi+lo+X-S >= thr.
This halves conv2/conv3 tensor columns vs the 2-pass hi/lo while keeping the
exact-product precision class. States are small integers -> exact in bf16.
"""
import numpy as np
import ml_dtypes

import concourse.bass as bass
import concourse.mybir as mybir
from concourse import tile
from concourse.bass_utils import run_bass_kernel_spmd
from concourse.tile_rust import add_dep_helper

F32 = mybir.dt.float32
BF16 = mybir.dt.bfloat16
BF = ml_dtypes.bfloat16
ALU = mybir.AluOpType
ACTF = mybir.ActivationFunctionType

NCORE = 8
NI = 8
T = 16
WP = 32                 # padded row stride (30 cols used; 64B-aligned bf16 rows)
HPAD = 30
PADPIX = HPAD * WP      # 960
NPIX = 784
NH = 392
TAPS = [(ky, kx) for ky in range(3) for kx in range(3)]


def _split_excess_waits(nc, maxw=1):
    """This walrus accepts only one sync-wait per instruction; hoist extras
    onto preceding same-engine nops."""
    ctr = [0]
    for bb in nc.m.functions[0].blocks:
        nl = []
        for inst in bb.instructions:
            w = list(inst.sync_info.on_wait) if inst.sync_info else []
            if len(w) > maxw:
                keep, exc = w[:maxw], w[maxw:]
                for cs in range(0, len(exc), maxw):
                    nop = mybir.InstNoOp(name=f"I-ws-{ctr[0]}", ins=[], outs=[])
                    ctr[0] += 1
                    nop.engine = inst.engine
                    nop.sync_info = mybir.SyncInfo(
                        on_wait=list(exc[cs:cs + maxw]), on_update=[])
                    nc.register_instruction(nop)
                    nl.append(nop)
                inst.sync_info = mybir.SyncInfo(
                    on_wait=list(keep), on_update=list(inst.sync_info.on_update))
            nl.append(inst)
        bb.instructions[:] = nl


def _rap(handle, offset, dims):
    """Raw access pattern on a DRAM tensor handle."""
    return bass.AP(handle, offset, [list(d) for d in dims])


def _pad3(ap_pad, nrow=128):
    """[P, 960] padded tile -> [P, 30, 32] view."""
    return ap_pad.rearrange("p (h w) -> p h w", w=WP)


def _interior(ap_pad):
    """[P, 960] padded tile -> [P, 28, 28] interior view."""
    return _pad3(ap_pad)[:, 1:29, 1:29]


def _cwin(ap_pad, ky, kx, h0):
    """Conv-tap window: out rows h0..h0+13 -> padded rows h0+ky.., cols kx.."""
    return _pad3(ap_pad)[:, h0 + ky:h0 + ky + 14, kx:kx + 28]


def _ihalf(ap_pad, e, h):
    """Interior half view [64, 14, 28] of image e in a padded pair tile."""
    v = ap_pad[64 * e:64 * e + 64, :].rearrange("p (h w) -> p h w", w=WP)
    return v[:, 1 + 14 * h:15 + 14 * h, 1:29]


def _pwin(ap_pad, e0, e1, dy, dx):
    """Pool window on partitions e0:e1: [P, 14, 14], elem (r,c) = padded
    (2r+1+dy, 2c+1+dx)."""
    v = ap_pad[e0:e1, :].rearrange("p (h w) -> p h w", w=WP)
    return v[:, 1 + dy:1 + dy + 28:2, 1 + dx:1 + dx + 28:2]


def build_nc(nt=T, ni=NI):
    nc = bass.Bass()
    qn = ni // 4
    pn = ni // 2

    xst = nc.dram_tensor("xst", [ni * nt, 27, 2, NH], BF16, kind="ExternalInput")
    w1q = nc.dram_tensor("w1q", [128, 3, 128], BF16, kind="ExternalInput")
    w2m = nc.dram_tensor("w2m", [128, 9, 128], BF16, kind="ExternalInput")
    w3m = nc.dram_tensor("w3m", [128, 9, 128], BF16, kind="ExternalInput")
    idq = nc.dram_tensor("idq", [128, 64], BF16, kind="ExternalInput")
    idxq = nc.dram_tensor("idxq", [128, 128], BF16, kind="ExternalInput")
    idsq = nc.dram_tensor("idsq", [128, 128], BF16, kind="ExternalInput")
    wf1s = nc.dram_tensor("wf1s", [98, 2, 128, 128], BF16, kind="ExternalInput")
    wf2s = nc.dram_tensor("wf2s", [128, 2, 10], BF16, kind="ExternalInput")
    thrL1 = nc.dram_tensor("thrL1", [128, nt], F32, kind="ExternalInput")
    thrH1 = nc.dram_tensor("thrH1", [128, nt], F32, kind="ExternalInput")
    thrH2 = nc.dram_tensor("thrH2", [128, nt], F32, kind="ExternalInput")
    thrF1 = nc.dram_tensor("thrF1", [128, nt], F32, kind="ExternalInput")
    bf2t = nc.dram_tensor("bf2t", [10, 1], F32, kind="ExternalInput")
    out = nc.dram_tensor("out", [ni, 10], F32, kind="ExternalOutput")
    sh3d = nc.dram_tensor("sh3d", [64, 196, ni, nt], BF16, kind="Internal")

    with tile.TileContext(nc) as tc:
        with (
            tc.tile_pool(name="wpool", bufs=1) as wp,
            tc.tile_pool(name="state", bufs=1) as stp,
            tc.tile_pool(name="scratch", bufs=1) as scp,
            tc.tile_pool(name="psum", bufs=1, space="PSUM") as psp,
        ):
            # ---- weights / thresholds ----
            w1s = wp.tile([128, 3, 128], BF16, name="w1s")
            nc.sync.dma_start(w1s[:, :, :], w1q[:, :, :])
            w2s = wp.tile([128, 9, 128], BF16, name="w2s")
            nc.sync.dma_start(w2s[:, :, :], w2m[:, :, :])
            w3s = wp.tile([128, 9, 128], BF16, name="w3s")
            nc.sync.dma_start(w3s[:, :, :], w3m[:, :, :])
            ids = wp.tile([128, 64], BF16, name="ids")
            nc.sync.dma_start(ids[:, :], idq[:, :])
            idxs = wp.tile([128, 128], BF16, name="idxs")
            nc.sync.dma_start(idxs[:, :], idxq[:, :])
            idss = wp.tile([128, 128], BF16, name="idss")
            nc.sync.dma_start(idss[:, :], idsq[:, :])
            wf2l = wp.tile([128, 2, 10], BF16, name="wf2l")
            nc.sync.dma_start(wf2l[:, :, :], wf2s[:, :, :])
            tL1 = wp.tile([128, nt], F32, name="tL1")
            nc.sync.dma_start(tL1[:, :], thrL1[:, :])
            tH1 = wp.tile([128, nt], F32, name="tH1")
            nc.sync.dma_start(tH1[:, :], thrH1[:, :])
            tH2 = wp.tile([128, nt], F32, name="tH2")
            nc.sync.dma_start(tH2[:, :], thrH2[:, :])
            tF1 = wp.tile([128, nt], F32, name="tF1")
            nc.sync.dma_start(tF1[:, :], thrF1[:, :])
            bf2l = wp.tile([10, 1], F32, name="bf2l")
            nc.sync.dma_start(bf2l[:, :], bf2t[:, :])
            wf1l = wp.tile([128, 2, 98, 128], BF16, name="wf1l")

            # ---- persistent states ----
            sL1 = []        # per image [128, 784]: rows 0-63 S_x1, 64-95 S_h0
            for i in range(ni):
                st = stp.tile([128, NPIX], BF16, name=f"sL1_{i}")
                nc.gpsimd.memset(st[:, :], 0.0)
                sL1.append(st)
            sH0q = []       # per quad, padded, img g at rows 32g (conv2 rhs)
            for q in range(qn):
                st = stp.tile([128, PADPIX], BF16, name=f"sH0q_{q}")
                nc.gpsimd.memset(st[:, :], 0.0)
                sH0q.append(st)
            sX1 = []        # per pair, padded, img e at rows 64e (x1 spikes)
            for p in range(pn):
                st = stp.tile([128, PADPIX], BF16, name=f"sX1_{p}")
                nc.gpsimd.memset(st[:, :], 0.0)
                sX1.append(st)
            sH1, sH2, qH3 = [], [], []
            for p in range(pn):
                s1 = stp.tile([128, PADPIX], BF16, name=f"sH1_{p}")
                nc.gpsimd.memset(s1[:, :], 0.0)
                sH1.append(s1)
                s2 = stp.tile([128, PADPIX], BF16, name=f"sH2_{p}")
                nc.gpsimd.memset(s2[:, :], 0.0)
                sH2.append(s2)
                q3 = stp.tile([128, 196, nt + 1], BF16, name=f"qH3_{p}")
                nc.gpsimd.memset(q3[:, :, :], 1.0)   # Q = S + 1
                qH3.append(q3)
            rtS = stp.tile([128, 98, ni * nt], BF16, name="rtS")

            last_in_bank = {}  # chain-key -> last matmul inst of prior chain

            def chain_dep(key, first_mm):
                if key in last_in_bank:
                    add_dep_helper(first_mm.ins, last_in_bank[key].ins,
                                   sync=False, reason="psum group order")

            FdL = {}
            Fd1 = {}
            Fd2 = {}
            Fd3 = {}

            def emit_wave(chains):
                n = max(len(c) for c in chains)
                for j in range(n):
                    for c in chains:
                        if j < len(c):
                            fw, lhs, rhs, st, sp, tp, key = c[j]
                            mm = nc.tensor.matmul(fw, lhs, rhs, start=st,
                                                  stop=sp, tile_position=tp)
                            if st:
                                chain_dep(key, mm)
                            if sp:
                                last_in_bank[key] = mm

            def stage_l1(t, q):
                imgs = [4 * q + g for g in range(4)]
                # ---------- L1 ----------
                imst = scp.tile([128, NPIX], BF16, tag="imst",
                                name=f"imst_{t}_{q}", bufs=3)
                for g, i in enumerate(imgs):
                    off = (i * nt + t) * 27 * 2 * NH
                    nc.sync.dma_start(imst[32 * g:32 * g + 27, :],
                                      _rap(xst, off, [[2 * NH, 27], [1, 2 * NH]]))
                fLs = FdL
                l1_mms = {i: [] for i in imgs}
                for g, i in enumerate(imgs):
                    for h in range(2):
                        fLs[(i, h)] = psp.tile([128, 512], F32, tag="F",
                                               name=f"fL_{t}_{i}_{h}", bufs=7)
                        fw = fLs[(i, h)][:, 0:NH]
                        rh = imst[32 * g:32 * g + 27, NH * h:NH * h + NH]
                        for v in range(3):
                            l1_mms[i].append(
                                (fw, w1s[32 * g:32 * g + 27, v, :], rh,
                                 v == 0, v == 2, (32 * g, 0), ("L", i, h)))
                # round-robin across images for row-group diversity
                for j in range(6):
                    for g, i in enumerate(imgs):
                        fw, lhs, rh, st, sp, tp, key = l1_mms[i][j]
                        mm = nc.tensor.matmul(fw, lhs, rh, start=st, stop=sp,
                                              tile_position=tp)
                        if st:
                            chain_dep(key, mm)
                        if sp:
                            last_in_bank[key] = mm
                for g, i in enumerate(imgs):
                    sL = scp.tile([128, NPIX], BF16, tag="s",
                                  name=f"s_{t}_{i}", bufs=8)
                    for h in range(2):
                        nc.vector.scalar_tensor_tensor(
                            sL[:, NH * h:NH * h + NH],
                            fLs[(i, h)][:, 0:NH],
                            tL1[:, t:t + 1],
                            sL1[i][:, NH * h:NH * h + NH],
                            op0=ALU.subtract, op1=ALU.is_ge)
                    nc.vector.tensor_tensor(
                        sL1[i][:, :], sL1[i][:, :], sL[:, :], ALU.add)
                    # maintain padded quad h0 copy on ScalarE (partition shift)
                    nc.scalar.copy(
                        _interior(sH0q[q][32 * g:32 * g + 32, :]),
                        sL1[i][64:96, :].rearrange("p (h w) -> p h w", w=28))
                    # maintain padded x1 pair copy (img e at rows 64e)
                    p_ = i // 2
                    e_ = i % 2
                    nc.scalar.copy(
                        _interior(sX1[p_][64 * e_:64 * e_ + 64, :]),
                        sL1[i][0:64, :].rearrange("p (h w) -> p h w", w=28))

            def stage_h1_mm(t, q):
                # ---------- h1: conv2(S_h0) hi|-lo in M + X - S on PE ----------
                f1s = Fd1
                for pp in range(2):
                    for e in range(2):
                        i = 4 * q + 2 * pp + e
                        for h in range(2):
                            f1s[(i, h)] = psp.tile([128, 512], F32, tag="F",
                                                   name=f"f1_{t}_{i}_{h}",
                                                   bufs=7)

                def h1_chain_list(pp, h, e):
                    p = 2 * q + pp
                    i = 4 * q + 2 * pp + e
                    g = 2 * pp + e
                    fw = f1s[(i, h)][:, 0:NH]
                    key = ("h1", i, h)
                    mms = [
                        (fw, idxs[64 * e:64 * e + 64, :], _ihalf(sX1[p], e, h),
                         True, False, (64 * e, 0), key),
                        (fw, idss[64 * e:64 * e + 64, :], _ihalf(sH1[p], e, h),
                         False, False, (64 * e, 0), key),
                    ]
                    for k, (ky, kx) in enumerate(TAPS):
                        rhs = _cwin(sH0q[q][32 * g:32 * g + 32, :], ky, kx,
                                    14 * h)
                        mms.append((fw, w2s[32 * g:32 * g + 32, k, :], rhs,
                                    False, k == 8, (32 * g, 0), key))
                    return mms

                waves = ([h1_chain_list(0, 0, 0), h1_chain_list(0, 1, 1),
                          h1_chain_list(1, 0, 0), h1_chain_list(1, 1, 1)],
                         [h1_chain_list(0, 0, 1), h1_chain_list(0, 1, 0),
                          h1_chain_list(1, 0, 1), h1_chain_list(1, 1, 0)])

                return waves

            def stage_h1_post(t, q):
                f1s = Fd1
                for pp in range(2):
                    p = 2 * q + pp
                    sP = scp.tile([128, NPIX], BF16, tag="s",
                                  name=f"sh1_{t}_{p}", bufs=8)
                    for e in range(2):
                        i = 4 * q + 2 * pp + e
                        lo = 64 * (1 - e)
                        hi = 64 * e
                        for h in range(2):
                            D = scp.tile([128, NH], F32, tag="d",
                                         name=f"d1_{t}_{i}_{h}", bufs=8)
                            nc.scalar.activation(
                                D[hi:hi + 64, :], f1s[(i, h)][lo:lo + 64, 0:NH],
                                ACTF.Identity, bias=tH1[hi:hi + 64, t:t + 1],
                                scale=1.0)
                            nc.vector.scalar_tensor_tensor(
                                sP[hi:hi + 64, NH * h:NH * h + NH],
                                f1s[(i, h)][hi:hi + 64, 0:NH],
                                0.0,
                                D[hi:hi + 64, :],
                                op0=ALU.add, op1=ALU.is_ge)
                    nc.vector.tensor_tensor(
                        _interior(sH1[p][:, :]), _interior(sH1[p][:, :]),
                        sP[:, :].rearrange("p (h w) -> p h w", w=28), ALU.add)

            def stage_h2_mm(t, q):
                # ---------- h2: conv3(S_h1) hi|-lo in M, - S on PE ----------
                f2s = Fd2
                for pp in range(2):
                    for e in range(2):
                        i = 4 * q + 2 * pp + e
                        for h in range(2):
                            f2s[(i, h)] = psp.tile([128, 512], F32, tag="F",
                                                   name=f"f2_{t}_{i}_{h}",
                                                   bufs=7)

                def h2_chain_list(pp, h, e):
                    p = 2 * q + pp
                    i = 4 * q + 2 * pp + e
                    fw = f2s[(i, h)][:, 0:NH]
                    key = ("h2", i, h)
                    mms = [(fw, idss[64 * e:64 * e + 64, :],
                            _ihalf(sH2[p], e, h),
                            True, False, (64 * e, 0), key)]
                    for k, (ky, kx) in enumerate(TAPS):
                        rhs = _cwin(sH1[p][64 * e:64 * e + 64, :], ky, kx,
                                    14 * h)
                        mms.append((fw, w3s[64 * e:64 * e + 64, k, :], rhs,
                                    False, k == 8, (64 * e, 0), key))
                    return mms

                waves = ([h2_chain_list(0, 0, 0), h2_chain_list(0, 1, 1),
                          h2_chain_list(1, 0, 0), h2_chain_list(1, 1, 1)],
                         [h2_chain_list(0, 0, 1), h2_chain_list(0, 1, 0),
                          h2_chain_list(1, 0, 1), h2_chain_list(1, 1, 0)])

                return waves

            def stage_h2_post(t, q):
                f2s = Fd2
                for pp in range(2):
                    p = 2 * q + pp
                    sP = scp.tile([128, NPIX], BF16, tag="s",
                                  name=f"sh2_{t}_{p}", bufs=8)
                    for e in range(2):
                        i = 4 * q + 2 * pp + e
                        lo = 64 * (1 - e)
                        hi = 64 * e
                        for h in range(2):
                            D = scp.tile([128, NH], F32, tag="d",
                                         name=f"d2_{t}_{i}_{h}", bufs=8)
                            nc.scalar.activation(
                                D[hi:hi + 64, :], f2s[(i, h)][lo:lo + 64, 0:NH],
                                ACTF.Identity, bias=tH2[hi:hi + 64, t:t + 1],
                                scale=1.0)
                            nc.vector.scalar_tensor_tensor(
                                sP[hi:hi + 64, NH * h:NH * h + NH],
                                f2s[(i, h)][hi:hi + 64, 0:NH],
                                0.0,
                                D[hi:hi + 64, :],
                                op0=ALU.add, op1=ALU.is_ge)
                    nc.vector.tensor_tensor(
                        _interior(sH2[p][:, :]), _interior(sH2[p][:, :]),
                        sP[:, :].rearrange("p (h w) -> p h w", w=28), ALU.add)

            def stage_h3(t, q):
                # ---------- h3: 2x2 sum pool on DVE (sums <=64, bf16-exact;
                # x1.1 applied in the compare, identical decisions) ----------
                for pp in range(2):
                    p = 2 * q + pp
                    v = _pad3(sH2[p][:, :])
                    pooled = scp.tile([128, 196], BF16, tag="pool",
                                      name=f"pl_{t}_{p}", bufs=4)
                    pv = pooled[:, :].rearrange("p (a b) -> p a b", b=14)
                    nc.vector.tensor_tensor(
                        pv, v[:, 1:29:2, 1:29:2], v[:, 1:29:2, 2:30:2],
                        ALU.add)
                    nc.vector.tensor_tensor(
                        pv, pv, v[:, 2:30:2, 1:29:2], ALU.add)
                    nc.vector.tensor_tensor(
                        pv, pv, v[:, 2:30:2, 2:30:2], ALU.add)
                    sP = scp.tile([128, NPIX], BF16, tag="s",
                                  name=f"sh3_{t}_{p}", bufs=8)
                    nc.vector.scalar_tensor_tensor(
                        sP[:, 0:196], pooled[:, :], 1.1,
                        qH3[p][:, :, t], op0=ALU.mult, op1=ALU.is_ge)
                    nc.vector.tensor_tensor(
                        qH3[p][:, :, t + 1], qH3[p][:, :, t],
                        sP[:, 0:196], ALU.add)
                    for e in range(2):
                        col = (2 * p + e) * nt + t
                        for j in range(2):
                            nc.scalar.copy(
                                rtS[64 * j:64 * j + 64, :, col:col + 1],
                                qH3[p][64 * e:64 * e + 64, j:196:2,
                                       t + 1:t + 2])

            # ================= time loop: L1(t+1) interleaves into t ====
            for q in range(qn):
                stage_l1(0, q)
            for t in range(nt):
                if t == 1:
                    for hl in range(2):
                        nc.sync.dma_start(
                            wf1l[:, hl, :, :],
                            _rap(wf1s, hl * 128 * 128,
                                 [[128, 128], [2 * 128 * 128, 98], [1, 128]]))
                for q in range(qn):
                    for wv in stage_h1_mm(t, q):
                        emit_wave(wv)
                    stage_h1_post(t, q)
                if t + 1 < nt:
                    for q in range(qn):
                        stage_l1(t + 1, q)
                for q in range(qn):
                    for wv in stage_h2_mm(t, q):
                        emit_wave(wv)
                    stage_h2_post(t, q)
                for q in range(qn):
                    stage_h3(t, q)

            # ---------- fc1 (rhs staged in SBUF during the loop) ----------
            ff1 = psp.tile([128, 512], F32, tag="F3", name="ff1", bufs=1)
            for px in range(98):
                for hl in range(2):
                    st_ = (px == 0 and hl == 0)
                    sp_ = (px == 97 and hl == 1)
                    nc.tensor.matmul(ff1[0:64, 0:ni * nt],
                                     wf1l[:, hl, px, 0:64], rtS[:, px, :],
                                     start=st_, stop=sp_,
                                     tile_position=(0, 0),
                                     skip_group_check=True)
                    nc.tensor.matmul(ff1[64:128, 0:ni * nt],
                                     wf1l[:, hl, px, 64:128], rtS[:, px, :],
                                     start=st_, stop=sp_,
                                     tile_position=(0, 64),
                                     skip_group_check=True)

            # f1 spike scan (Q_h3 includes +1 offset; folded into thrF1)
            sf1 = stp.tile([128, ni], BF16, name="sf1")
            nc.gpsimd.memset(sf1[:, :], 0.0)
            for t in range(nt):
                sPf = scp.tile([128, ni], BF16, tag="sf", name=f"sf_{t}", bufs=2)
                nc.vector.scalar_tensor_tensor(
                    sPf[:, :],
                    ff1[:, 0:ni * nt].rearrange("p (n t) -> p n t", t=nt)[:, :, t],
                    tF1[:, t:t + 1], sf1[:, :],
                    op0=ALU.subtract, op1=ALU.is_ge)
                nc.vector.tensor_tensor(sf1[:, :], sf1[:, :], sPf[:, :], ALU.add)

            # ---------- fc2 + readout ----------
            ff2 = psp.tile([128, 512], F32, tag="F3", name="ff2", bufs=1)
            for hl in range(2):
                nc.tensor.matmul(ff2[0:10, 0:ni], wf2l[:, hl, :], sf1[:, :],
                                 start=(hl == 0), stop=(hl == 1))
            osb = scp.tile([10, ni], F32, tag="osb", name="osb")
            nc.scalar.activation(osb[:, :], ff2[0:10, 0:ni], ACTF.Identity,
                                 bias=bf2l[:, :], scale=1.0 / nt)
            nc.sync.dma_start(out[:, :].rearrange("n o -> o n"), osb[:, :])

    _split_excess_waits(nc)
    return nc


# ---------------- host side ----------------

def _split(a):
    hi = np.asarray(a, np.float64).astype(BF)
    lo = (np.asarray(a, np.float64) - hi.astype(np.float64)).astype(BF)
    return hi, lo


def _split3(a):
    a = np.asarray(a, np.float64)
    p0 = a.astype(BF)
    r = a - p0.astype(np.float64)
    p1 = r.astype(BF)
    p2 = (r - p1.astype(np.float64)).astype(BF)
    return p0, p1, p2


def _prep_shared(w1a, b1a, w1, b1, w2, b2, w3, b3, wf1, bf1, wf2, bf2, nt=T):
    d = {}
    # L1 lhsT [9, 128]: cols 0-63 w1a, 64-95 w1, 96-127 zero; rows k=3ky+kx
    # 3-term split W = W0+W1+W2; K=27 stacked variants to cover products with
    # the 3-term x split (X0,X1,X2): [W0;W1;W2], [W1;W0;0], [W2;0;W0]
    l1 = np.zeros((9, 128), np.float64)
    l1[:, 0:64] = w1a.reshape(64, 9).T
    l1[:, 64:96] = w1.reshape(32, 9).T
    W0, W1, W2 = _split3(l1)
    Z = np.zeros_like(W0)
    variants = [np.concatenate([W0, W1, W2]), np.concatenate([W1, W0, Z]),
                np.concatenate([W2, Z, W0])]
    w1qa = np.zeros((128, 3, 128), BF)
    for g in range(4):
        for v in range(3):
            w1qa[32 * g:32 * g + 27, v, :] = variants[v]
    d["w1q"] = w1qa
    # conv2 [tap, c, m]: hi at cols 0-63, -lo at cols 64-127, 4 row bases
    a2 = np.transpose(w2.reshape(64, 32, 3, 3), (2, 3, 1, 0)).reshape(9, 32, 64)
    h2v, l2v = _split(a2)
    w2ma = np.zeros((128, 9, 128), BF)
    for g in range(4):
        e = g % 2
        hi, lo = 64 * e, 64 * (1 - e)
        w2ma[32 * g:32 * g + 32, :, hi:hi + 64] = np.transpose(h2v, (1, 0, 2))
        w2ma[32 * g:32 * g + 32, :, lo:lo + 64] = -np.transpose(l2v, (1, 0, 2))
    d["w2m"] = w2ma
    a3 = np.transpose(w3.reshape(64, 64, 3, 3), (2, 3, 1, 0)).reshape(9, 64, 64)
    h3v, l3v = _split(a3)
    w3ma = np.zeros((128, 9, 128), BF)
    for e in range(2):
        hi, lo = 64 * e, 64 * (1 - e)
        w3ma[64 * e:64 * e + 64, :, hi:hi + 64] = np.transpose(h3v, (1, 0, 2))
        w3ma[64 * e:64 * e + 64, :, lo:lo + 64] = -np.transpose(l3v, (1, 0, 2))
    d["w3m"] = w3ma
    idqa = np.zeros((128, 64), BF)
    for e in range(2):
        idqa[64 * e:64 * e + 64, :] = np.eye(64, dtype=BF)
    d["idq"] = idqa
    idxa = np.zeros((128, 128), BF)
    idsa = np.zeros((128, 128), BF)
    for e in range(2):
        idxa[64 * e:64 * e + 64, 64 * e:64 * e + 64] = np.eye(64, dtype=BF)
        idsa[64 * e:64 * e + 64, 64 * e:64 * e + 64] = -np.eye(64, dtype=BF)
    d["idxq"] = idxa
    d["idsq"] = idsa
    # fc1 tiles [pxh, hl, row=64j+c, m]; fc1 consumes h3 spikes (no 1.1)
    wf1p = np.asarray(wf1, np.float64)                  # [128, 64, 14, 14]
    wf1f = wf1p.reshape(128, 64, 196)                   # px = 14*h + w
    wf1sa = np.zeros((98, 2, 128, 128), BF)
    hi1, lo1 = _split(wf1f)
    for pxh in range(98):
        for j in range(2):
            px = 2 * pxh + j
            wf1sa[pxh, 0, 64 * j:64 * j + 64, :] = hi1[:, :, px].T
            wf1sa[pxh, 1, 64 * j:64 * j + 64, :] = lo1[:, :, px].T
    d["wf1s"] = wf1sa
    h2v_, l2v_ = _split(np.asarray(wf2, np.float64).T)    # [128, 10]
    wf2sa = np.zeros((128, 2, 10), BF)
    wf2sa[:, 0, :] = h2v_
    wf2sa[:, 1, :] = l2v_
    d["wf2s"] = wf2sa
    # thresholds [128, nt]
    tsteps = np.arange(1, nt + 1)
    thrL1 = np.ones((128, nt), np.float32)
    thrL1[0:64] = 1.0 - tsteps[None, :] * np.asarray(b1a, np.float64)[:, None]
    thrL1[64:96] = 1.0 - tsteps[None, :] * np.asarray(b1, np.float64)[:, None]
    d["thrL1"] = thrL1
    thrH1 = np.ones((128, nt), np.float32)
    b2d = np.asarray(b2, np.float64)
    thrH1[0:64] = 1.0 - tsteps[None, :] * b2d[:, None]
    thrH1[64:128] = 1.0 - tsteps[None, :] * b2d[:, None]
    d["thrH1"] = thrH1
    thrH2 = np.ones((128, nt), np.float32)
    b3d = np.asarray(b3, np.float64)
    thrH2[0:64] = 1.0 - tsteps[None, :] * b3d[:, None]
    thrH2[64:128] = 1.0 - tsteps[None, :] * b3d[:, None]
    d["thrH2"] = thrH2
    # f1: (F_comp = wf1p . Q) >= S + 1 + rowsum(wf1p) - (t+1)*bf1
    rs = wf1f.sum(axis=(1, 2))                          # [128]
    thrF1 = np.zeros((128, nt), np.float32)
    thrF1[:, :] = (1.0 + rs[:, None]
                   - tsteps[None, :] * np.asarray(bf1, np.float64)[:, None])
    d["thrF1"] = thrF1
    d["bf2t"] = np.asarray(bf2, np.float32).reshape(10, 1)
    return d


def _prep_x(xc, nt=T):
    """xc [ni, 1, 28, 28, nt] -> host im2col of the 3-term-split cumulative
    input, stacked [ni*nt, 27, 2, 392] = ([X0;X1;X2], half, px)."""
    ni = xc.shape[0]
    X = np.cumsum(np.asarray(xc, np.float64), axis=-1)[:, 0]   # [ni, 28, 28, nt]
    X = np.moveaxis(X, -1, 1)                                  # [ni, nt, 28, 28]
    pad = np.zeros((ni * nt, 30, 30), np.float64)
    pad[:, 1:29, 1:29] = X.reshape(ni * nt, 28, 28)
    col = np.zeros((ni * nt, 9, 2, NH), np.float64)
    for k, (ky, kx) in enumerate(TAPS):
        for h in range(2):
            col[:, k, h, :] = pad[:, 14 * h + ky:14 * h + ky + 14,
                                  kx:kx + 28].reshape(ni * nt, NH)
    x0, x1, x2 = _split3(col)
    return np.concatenate([x0, x1, x2], axis=1)  # [ni*nt, 27, 2, NH]


_NC_CACHE = {}


def kernel(x, w1a, b1a, w1, b1, w2, b2, w3, b3, wf1, bf1, wf2, bf2):
    x = np.asarray(x)
    n_total = x.shape[0]
    ni = n_total // NCORE
    key = (ni, T)
    if key not in _NC_CACHE:
        _NC_CACHE[key] = build_nc(T, ni)
    nc = _NC_CACHE[key]

    shared = _prep_shared(w1a, b1a, w1, b1, w2, b2, w3, b3, wf1, bf1, wf2, bf2)
    in_maps = []
    for c in range(NCORE):
        m = dict(shared)
        m["xst"] = _prep_x(x[c * ni:(c + 1) * ni])
        in_maps.append(m)

    res = run_bass_kernel_spmd(nc, in_maps, list(range(NCORE))).results
    return np.concatenate([res[c]["out"] for c in range(NCORE)], axis=0)
